# revision 1
# baseline (speedup 1.0000x reference)
"""HPSS (harmonic/percussive source separation) Trainium2 kernel.

Input S [2,2,1025,1024] f32. Per (b,c) plane: harm = median-31 along W
(zero-padded), perc = median-31 along H; softmask with power=2, margin=1;
returns (S*mask_h, S*mask_p).

Sharding: 8 cores = 4 planes x 2 W-halves. Each core computes, for its
plane/half: perc medians for its 512 columns over all 1025 rows, harm
medians + softmask outputs for rows 0..1023 x its 512 columns. Row 1024
(one row per plane) is finished on the host (harm median of 4 rows +
softmask using the device-computed perc median).

Algorithm (exact median of 31): decompose the padded filter axis into
31-blocks; per block compute running order statistics (levels 1..16) as
prefix scans (tensor_tensor_scan min with add-1e30 reset masks) and the
same on the reversed axis (suffix stats). The median of window
[t..t+30] = min over layers l=0..16 of max(suf_l[block i, slot o],
pre_{16-l}[block i+1, slot o-1]) with t = 31*i + o; out-of-range list
positions read the guard value 2.0 (> any data), making all layer APs
uniform.
"""
import sys

import numpy as np

sys.path.insert(0, "/opt/trn_rl_repo")

P = 128
K = 31
LEV = 16
GUARD = 2.0
NB_H = 18           # harm blocks per 558-col local strip
NH = NB_H * K       # 558
NQ = 2              # q-rows per harm chunk (256 rows)
NHC = NQ * NH       # 1116 flat scan length per harm chunk
NB_P = 35           # perc blocks (1085 padded H)
NP = NB_P * K       # 1085
HALF = 15

_PROGRAM = None


def _build_program():
    from contextlib import ExitStack

    import concourse.mybir as mybir
    import concourse.tile as tile
    from concourse import bacc
    from concourse.tile import add_dep_helper

    f32 = mybir.dt.float32
    MIN = mybir.AluOpType.min
    MAX = mybir.AluOpType.max
    ADD = mybir.AluOpType.add
    MULT = mybir.AluOpType.mult

    nc = bacc.Bacc("TRN2", target_bir_lowering=False, debug=True)
    XH = nc.declare_dram_parameter("XH", [1025, NH], f32, isOutput=False)
    XP = nc.declare_dram_parameter("XP", [512, NP], f32, isOutput=False)
    ID = nc.declare_dram_parameter("ID", [P, P], f32, isOutput=False)
    PM = nc.declare_dram_parameter("PM", [512, NP], f32, isOutput=True)
    OH = nc.declare_dram_parameter("OH", [1024, 512], f32, isOutput=True)
    OP = nc.declare_dram_parameter("OP", [1024, 512], f32, isOutput=True)

    from bass_rust import ActivationFunctionType as AF

    def median_chunk(ctx, tc, pool, x, N, nb, mask, tag):
        """x: 2-D AP [P, N] (N = nb*K). Returns cmin tile [P, N] with
        cmin[31*i + o] = median of x[31*i+o .. +30] for i <= nb-2."""
        pre = [pool.tile([P, N], f32, tag=f"mc_pre{l}", name=f"mc_pre{l}") for l in range(LEV)]
        suf = [pool.tile([P, N], f32, tag=f"mc_suf{l}", name=f"mc_suf{l}") for l in range(LEV)]
        t = pool.tile([P, N], f32, tag="mc_t", name="mc_t")
        t3 = t[:].rearrange("p (b k) -> p b k", k=K)
        x3 = x.rearrange("p (b k) -> p b k", k=K)

        # prefix side: level 1 scans x directly; t's j==0 slots stay 2.0
        nc.vector.memset(t3[:, :, 0:1], GUARD)
        nc.vector.tensor_tensor_scan(pre[0][:], mask, x, GUARD,
                                     op0=ADD, op1=MIN)
        for l in range(1, LEV):
            s3 = pre[l - 1][:].rearrange("p (b k) -> p b k", k=K)
            nc.vector.tensor_tensor(t3[:, :, 1:K], s3[:, :, 0:K - 1],
                                    x3[:, :, 1:K], op=MAX)
            nc.vector.tensor_tensor_scan(pre[l][:], mask, t[:], GUARD,
                                         op0=ADD, op1=MIN)
        # suffix side: reversed scans; t's j==30 slots stay 2.0
        nc.vector.memset(t3[:, :, K - 1:K], GUARD)
        nc.vector.tensor_tensor_scan(suf[0][:, ::-1], mask, x[:, ::-1],
                                     GUARD, op0=ADD, op1=MIN)
        for l in range(1, LEV):
            s3 = suf[l - 1][:].rearrange("p (b k) -> p b k", k=K)
            nc.vector.tensor_tensor(t3[:, :, 0:K - 1], s3[:, :, 1:K],
                                    x3[:, :, 0:K - 1], op=MAX)
            nc.vector.tensor_tensor_scan(suf[l][:, ::-1], mask, t[:, ::-1],
                                         GUARD, op0=ADD, op1=MIN)

        cmin = pool.tile([P, N], f32, tag="mc_cmin", name="mc_cmin")
        nc.vector.memset(cmin[:], GUARD)
        ic = nb - 1  # merge block count: i in [0, nb-2], i+1 <= nb-1
        for lay in range(LEV + 1):
            o_lo = max(0, 16 - lay)
            o_hi = min(30, 31 - lay)
            w = o_hi - o_lo + 1
            cm = cmin[:].rearrange("p (b k) -> p b k", k=K)[:, 0:ic, o_lo:o_lo + w]
            if lay == 0:
                b = pre[15][:].rearrange("p (b k) -> p b k", k=K)
                b = b[:, 1:1 + ic, o_lo - 1:o_lo - 1 + w]
                nc.vector.tensor_tensor(cm, cm, b, op=MIN)
            elif lay == 16:
                a = suf[15][:].rearrange("p (b k) -> p b k", k=K)
                a = a[:, 0:ic, o_lo:o_lo + w]
                nc.vector.tensor_tensor(cm, cm, a, op=MIN)
            else:
                a = suf[lay - 1][:].rearrange("p (b k) -> p b k", k=K)
                a = a[:, 0:ic, o_lo:o_lo + w]
                b = pre[16 - lay - 1][:].rearrange("p (b k) -> p b k", k=K)
                b = b[:, 1:1 + ic, o_lo - 1:o_lo - 1 + w]
                tm = t3[:, 0:ic, o_lo:o_lo + w]
                nc.vector.tensor_tensor(tm, a, b, op=MAX)
                nc.vector.tensor_tensor(cm, cm, tm, op=MIN)
        return cmin

    with tile.TileContext(nc) as tc:
        with ExitStack() as ctx:
            cpool = ctx.enter_context(tc.tile_pool(name="const", bufs=1))
            pool = ctx.enter_context(tc.tile_pool(name="work", bufs=1))
            ppool = ctx.enter_context(tc.tile_pool(name="ps", bufs=2, space="PSUM"))

            mask = cpool.tile([P, NHC], f32)
            nc.vector.memset(mask[:], 0.0)
            m3 = mask[:].rearrange("p (b k) -> p b k", k=K)
            nc.vector.memset(m3[:, :, 0:1], 1e30)
            ident = cpool.tile([P, P], f32)
            nc.sync.dma_start(ident[:], ID[:])

            # ---- phase A: perc (median along H), 4 column-group chunks
            pm_writes = []
            for g in range(4):
                xp = pool.tile([P, NP], f32, tag="xin", name="xp")
                nc.sync.dma_start(xp[:], XP[g * P:(g + 1) * P, :])
                cmin = median_chunk(ctx, tc, pool, xp[:], NP, NB_P,
                                    mask[:, 0:NP], "pc")
                w = nc.sync.dma_start(PM[g * P:(g + 1) * P, :], cmin[:])
                pm_writes.append(w)

            # ---- phase B: harm (median along W) + softmask, 4 row chunks
            for kk in range(4):
                xh = pool.tile([P, NQ * NH], f32, tag="xin", name="xh")
                xh3 = xh[:].rearrange("p (q n) -> p q n", n=NH)
                nc.sync.dma_start(
                    xh3, XH[0:1024, :].rearrange("(a q p) n -> a p q n", p=P, q=NQ)[kk])
                cmin = median_chunk(ctx, tc, pool, xh[:], NHC, NQ * NB_H,
                                    mask[:], "hc")

                # perc slices for rows of this chunk, via PE transpose
                percT = pool.tile([P, NQ, 512], f32, tag="percT", name="percT")
                for qq in range(NQ):
                    for cg in range(4):
                        pmt = pool.tile([P, P], f32, tag="pmt", name="pmt")
                        r = nc.sync.dma_start(
                            pmt[:],
                            PM[cg * P:(cg + 1) * P,
                               kk * 256 + qq * P: kk * 256 + (qq + 1) * P])
                        for w in pm_writes:
                            add_dep_helper(r.ins, w.ins, reason="PM readback after write")
                        ps = ppool.tile([P, P], f32, tag="ps", name="ps")
                        nc.tensor.transpose(ps[:], pmt[:], ident[:])
                        nc.scalar.copy(percT[:, qq, cg * P:(cg + 1) * P], ps[:])

                # softmask: oh = S*h^2/(h^2+p^2), op = S*p^2/(h^2+p^2)
                cm3 = cmin[:].rearrange("p (q n) -> p q n", n=NH)
                h = cm3[:, :, 0:512]
                s_in = xh3[:, :, HALF:HALF + 512]
                h2 = pool.tile([P, NQ, 512], f32, tag="h2", name="h2")
                den = pool.tile([P, NQ, 512], f32, tag="den", name="den")
                nc.scalar.activation(h2[:], h, AF.Square)
                nc.scalar.activation(percT[:], percT[:], AF.Square)
                nc.vector.tensor_tensor(den[:], h2[:], percT[:], op=ADD)
                nc.vector.reciprocal(den[:], den[:])
                nc.vector.tensor_tensor(h2[:], h2[:], den[:], op=MULT)
                nc.vector.tensor_tensor(percT[:], percT[:], den[:], op=MULT)
                nc.vector.tensor_tensor(h2[:], h2[:], s_in, op=MULT)
                nc.vector.tensor_tensor(percT[:], percT[:], s_in, op=MULT)
                oh_d = OH[:].rearrange("(a q p) n -> a p q n", p=P, q=NQ)[kk]
                op_d = OP[:].rearrange("(a q p) n -> a p q n", p=P, q=NQ)[kk]
                nc.sync.dma_start(oh_d, h2[:])
                nc.sync.dma_start(op_d, percT[:])

    nc.finalize()
    return nc


def _get_program():
    global _PROGRAM
    if _PROGRAM is None:
        _PROGRAM = _build_program()
    return _PROGRAM


def _host_prep(S):
    """Returns in_maps (8 dicts)."""
    ident = np.eye(P, dtype=np.float32)
    in_maps = []
    for c in range(8):
        pl, h = c >> 1, c & 1
        b, ch = pl >> 1, pl & 1
        Sp = S[b, ch]
        xh = np.zeros((1025, NH), np.float32)
        lo = 512 * h - HALF
        s0, s1 = max(0, lo), min(1024, lo + NH)
        xh[:, s0 - lo:s1 - lo] = Sp[:, s0:s1]
        xp = np.zeros((512, NP), np.float32)
        xp[:, HALF:HALF + 1025] = Sp[:, 512 * h:512 * h + 512].T
        in_maps.append({"XH": xh, "XP": xp, "ID": ident})
    return in_maps


def _median31_rows(rows):
    """Exact median-31 along last axis with zero pad; rows [R, 1024]."""
    R, W = rows.shape
    p = np.pad(rows, ((0, 0), (HALF, HALF)))
    win = np.lib.stride_tricks.sliding_window_view(p, K, axis=1)
    return np.median(win, axis=2).astype(np.float32)


def kernel(S):
    from concourse.bass_utils import run_bass_kernel_spmd

    S = np.asarray(S, np.float32)
    nc = _get_program()
    in_maps = _host_prep(S)
    res = run_bass_kernel_spmd(nc, in_maps, list(range(8)))

    out_h = np.empty_like(S)
    out_p = np.empty_like(S)
    perc_1024 = np.empty((2, 2, 1024), np.float32)
    for c in range(8):
        pl, h = c >> 1, c & 1
        b, ch = pl >> 1, pl & 1
        r = res.results[c]
        out_h[b, ch, 0:1024, 512 * h:512 * h + 512] = r["OH"]
        out_p[b, ch, 0:1024, 512 * h:512 * h + 512] = r["OP"]
        perc_1024[b, ch, 512 * h:512 * h + 512] = r["PM"][:, 1024]
    # host fallback: row 1024 of each plane
    rows = S[:, :, 1024, :].reshape(4, 1024)
    harm_1024 = _median31_rows(rows).reshape(2, 2, 1024)
    h2 = harm_1024 * harm_1024
    p2 = perc_1024 * perc_1024
    rden = 1.0 / (h2 + p2)
    out_h[:, :, 1024, :] = S[:, :, 1024, :] * h2 * rden
    out_p[:, :, 1024, :] = S[:, :, 1024, :] * p2 * rden
    return out_h, out_p



# revision 4
# speedup vs baseline: 1.9076x; 1.9076x over previous
"""HPSS (harmonic/percussive source separation) Trainium2 kernel, v2.

Input S [2,2,1025,1024] f32. Per (b,c) plane: harm = median-31 along W
(zero-padded), perc = median-31 along H; softmask with power=2, margin=1;
returns (S*mask_h, S*mask_p).

Sharding: 8 cores = 4 planes x 2 W-halves. Each core computes perc medians
for its 512 columns over rows 0..1024 and harm medians + softmask outputs
for rows 0..1023 x its 512 columns. Row 1024 is finished on the host.

v2 vs v1:
- bf16 end to end (DVE 2x_1p mode for tensor_tensor; half the DMA).
- Exact median-31 via prefix/suffix order statistics, but each level's
  scan/TT processes only the 16 slots per 31-block that the final merge
  can ever read (level-l prefix stats are only consumed at slots
  [l-1, l+14]), in compact per-level tiles: halves scan+TT elements.
- Input staged in DRAM twice (X32 and XODD, shifted one slot) so every
  strided bf16 tensor_tensor operand is 4-byte aligned (2x_1p eligible).
- Merge layers accumulate into parity-split cmin tiles (even layers ->
  cminA at slot o, odd layers -> cminB at slot o-1) so both stay aligned;
  one final min combines them, writing ragged-contiguous outputs.
- Perc medians stay in SBUF and are transposed with the PE (identity
  matmul) instead of a DRAM round trip.
- reciprocal_approx_fast instead of the 6.5us iterative reciprocal.
"""
import sys

import numpy as np

sys.path.insert(0, "/opt/trn_rl_repo")

P = 128
K = 31
KS = 32            # padded block stride in storage
LEV = 16
GUARD = 2.0
NB_P = 35          # perc blocks (covers padded-H 1085)
NB_H = 18          # harm blocks per q-row (covers 558-col strip)
NQ = 2             # q row-groups per harm chunk
NBH = NQ * NB_H    # 36
HALF = 15

_PROGRAM = None


def _build_program():
    from contextlib import ExitStack

    import concourse.mybir as mybir
    import concourse.tile as tile
    from concourse import bacc

    f32 = mybir.dt.float32
    bf16 = mybir.dt.bfloat16
    MIN = mybir.AluOpType.min
    MAX = mybir.AluOpType.max
    ADD = mybir.AluOpType.add
    MULT = mybir.AluOpType.mult
    SUB = mybir.AluOpType.subtract

    from bass_rust import ActivationFunctionType as AF

    nc = bacc.Bacc("TRN2", target_bir_lowering=False, debug=True)
    XP32 = nc.declare_dram_parameter("XP32", [512, NB_P * KS], bf16, isOutput=False)
    XPODD = nc.declare_dram_parameter("XPODD", [512, NB_P * KS], bf16, isOutput=False)
    XH32 = nc.declare_dram_parameter("XH32", [1024, NB_H * KS], bf16, isOutput=False)
    XHODD = nc.declare_dram_parameter("XHODD", [1024, NB_H * KS], bf16, isOutput=False)
    XS = nc.declare_dram_parameter("XS", [1024, 512], bf16, isOutput=False)
    ID = nc.declare_dram_parameter("ID", [P, P], bf16, isOutput=False)
    OH = nc.declare_dram_parameter("OH", [1024, 512], bf16, isOutput=True)
    OP = nc.declare_dram_parameter("OP", [1024, 512], bf16, isOutput=True)
    PMR = nc.declare_dram_parameter("PMR", [512, 2], bf16, isOutput=True)

    W16MAX = NBH * 16  # 576

    with tile.TileContext(nc) as tc:
        with ExitStack() as ctx:
            cpool = ctx.enter_context(tc.tile_pool(name="const", bufs=1))
            inpool = ctx.enter_context(tc.tile_pool(name="in", bufs=2))
            pool = ctx.enter_context(tc.tile_pool(name="work", bufs=1))
            ppool = ctx.enter_context(tc.tile_pool(name="ps", bufs=2, space="PSUM"))

            mask = cpool.tile([P, W16MAX], bf16)
            nc.vector.memset(mask[:], 0.0)
            nc.vector.memset(
                mask[:].rearrange("p (b k) -> p b k", k=16)[:, :, 0:1], 1e30)
            ident = cpool.tile([P, P], bf16)
            nc.sync.dma_start(ident[:], ID[:])

            pre = [pool.tile([P, W16MAX], bf16, tag=f"pre{l}", name=f"pre{l}")
                   for l in range(LEV)]
            suf = [pool.tile([P, W16MAX], bf16, tag=f"suf{l}", name=f"suf{l}")
                   for l in range(LEV)]

            def median_chunk(nb, x32t, xoddt):
                """Computes cminA/cminB ([P, nb, KS] views) for blocks
                0..nb-2; returns (cA3, cB3) AP views."""
                W16 = nb * 16
                x3 = x32t[:].rearrange("p (b k) -> p b k", k=KS)
                xo3 = xoddt[:].rearrange("p (b k) -> p b k", k=KS)
                mk = mask[:, 0:W16]

                t = pool.tile([P, W16MAX], bf16, tag="t", name="t")
                t3 = t[:, 0:W16].rearrange("p (b k) -> p b k", k=16)
                xc = pool.tile([P, W16MAX], bf16, tag="xc", name="xc")
                xc3 = xc[:, 0:W16].rearrange("p (b k) -> p b k", k=16)

                # ---- prefix side: pre[l] compact slot k <-> logical l+k
                nc.scalar.copy(xc3, x3[:, :, 0:16])
                nc.vector.tensor_tensor_scan(
                    pre[0][:, 0:W16], mk, xc[:, 0:W16], GUARD, op0=ADD, op1=MIN)
                for l in range(1, LEV):
                    p3 = pre[l - 1][:, 0:W16].rearrange("p (b k) -> p b k", k=16)
                    if l % 2 == 0:
                        xv = x3[:, :, l:l + 16]
                    else:
                        xv = xo3[:, :, l - 1:l + 15]
                    nc.vector.tensor_tensor(t3, p3, xv, op=MAX)
                    nc.vector.tensor_tensor_scan(
                        pre[l][:, 0:W16], mk, t[:, 0:W16], GUARD,
                        op0=ADD, op1=MIN)

                # ---- suffix side: suf[l] compact slot k <-> logical 15-l+k
                nc.scalar.copy(xc3, x3[:, :, 15:31])
                nc.vector.tensor_tensor_scan(
                    suf[0][:, 0:W16][:, ::-1], mk, xc[:, 0:W16][:, ::-1],
                    GUARD, op0=ADD, op1=MIN)
                for l in range(1, LEV):
                    s3 = suf[l - 1][:, 0:W16].rearrange("p (b k) -> p b k", k=16)
                    if l % 2 == 1:
                        xv = x3[:, :, 15 - l:31 - l]
                    else:
                        xv = xo3[:, :, 14 - l:30 - l]
                    nc.vector.tensor_tensor(t3, s3, xv, op=MAX)
                    nc.vector.tensor_tensor_scan(
                        suf[l][:, 0:W16][:, ::-1], mk, t[:, 0:W16][:, ::-1],
                        GUARD, op0=ADD, op1=MIN)

                # ---- merge: cmin[o] = min over lay of
                #      max(suf_lay[b, o], pre_{16-lay}[b+1, o-1])
                nA = nb - 1
                cA = pool.tile([P, NBH * KS], bf16, tag="cA", name="cA")
                cB = pool.tile([P, NBH * KS], bf16, tag="cB", name="cB")
                cA3 = cA[:, 0:nb * KS].rearrange("p (b k) -> p b k", k=KS)
                cB3 = cB[:, 0:nb * KS].rearrange("p (b k) -> p b k", k=KS)
                nc.vector.memset(cB[:, 0:nb * KS], GUARD)
                s153 = suf[15][:, 0:W16].rearrange("p (b k) -> p b k", k=16)
                p153 = pre[15][:, 0:W16].rearrange("p (b k) -> p b k", k=16)
                # lay=16 (pure suffix, o in [0,15]) and lay=0 (pure prefix,
                # o in [16,30]) initialize cminA by copy.
                nc.scalar.copy(cA3[:, 0:nA, 0:16], s153[:, 0:nA, :])
                nc.scalar.copy(cA3[:, 0:nA, 16:31], p153[:, 1:nb, 0:15])
                tm = pool.tile([P, W16MAX], bf16, tag="tm", name="tm")
                tm3 = tm[:, 0:nA * 16].rearrange("p (b k) -> p b k", k=16)
                for lay in range(1, LEV):
                    a3 = suf[lay - 1][:, 0:W16].rearrange(
                        "p (b k) -> p b k", k=16)
                    b3 = pre[15 - lay][:, 0:W16].rearrange(
                        "p (b k) -> p b k", k=16)
                    nc.vector.tensor_tensor(
                        tm3, a3[:, 0:nA, :], b3[:, 1:nb, :], op=MAX)
                    if lay % 2 == 0:
                        dst = cA3[:, 0:nA, 16 - lay:32 - lay]
                    else:
                        dst = cB3[:, 0:nA, 15 - lay:31 - lay]
                    nc.vector.tensor_tensor(dst, dst, tm3, op=MIN)
                return cA3, cB3

            # ================= perc phase: 4 column-group chunks
            pcomp = [pool.tile([P, 1056], bf16, tag=f"pc{cg}", name=f"pc{cg}")
                     for cg in range(4)]
            for cg in range(4):
                xp = inpool.tile([P, NB_P * KS], bf16, tag="xp", name="xp")
                xpo = inpool.tile([P, NB_P * KS], bf16, tag="xpo", name="xpo")
                nc.sync.dma_start(xp[:], XP32[cg * P:(cg + 1) * P, :])
                nc.sync.dma_start(xpo[:], XPODD[cg * P:(cg + 1) * P, :])
                cA3, cB3 = median_chunk(NB_P, xp, xpo)
                nA = NB_P - 1
                pc3 = pcomp[cg][:, 0:nA * K].rearrange("p (b s) -> p b s", s=K)
                nc.vector.tensor_tensor(
                    pc3[:, :, 1:31], cA3[:, 0:nA, 1:31], cB3[:, 0:nA, 0:30],
                    op=MIN)
                nc.scalar.copy(pc3[:, :, 0:1], cA3[:, 0:nA, 0:1])
                nc.sync.dma_start(PMR[cg * P:(cg + 1) * P, :],
                                  pcomp[cg][:, 1024:1026])

            # ================= harm phase: 4 row chunks of [128, NQ*576]
            for kk in range(4):
                xh = inpool.tile([P, NBH * KS], bf16, tag="xh", name="xh")
                xho = inpool.tile([P, NBH * KS], bf16, tag="xho", name="xho")
                xs = inpool.tile([P, NQ * 512], bf16, tag="xs", name="xs")
                xh3 = xh[:].rearrange("p (q n) -> p q n", q=NQ)
                xho3 = xho[:].rearrange("p (q n) -> p q n", q=NQ)
                xs3 = xs[:].rearrange("p (q n) -> p q n", q=NQ)
                nc.sync.dma_start(
                    xh3, XH32[:].rearrange("(a q p) n -> a p q n", p=P, q=NQ)[kk])
                nc.sync.dma_start(
                    xho3, XHODD[:].rearrange("(a q p) n -> a p q n", p=P, q=NQ)[kk])
                nc.sync.dma_start(
                    xs3, XS[:].rearrange("(a q p) n -> a p q n", p=P, q=NQ)[kk])

                cA3, cB3 = median_chunk(NBH, xh, xho)

                # ragged-contiguous harm medians per q: hc[q, 31*b+s]
                hc = pool.tile([P, NQ * 527], bf16, tag="hc", name="hc")
                hc4 = hc[:].rearrange("p (q l) -> p q l", q=NQ)
                hc5 = hc4.rearrange("p q (b s) -> p q b s", s=K)
                cA4 = cA3.rearrange("p (q b) k -> p q b k", q=NQ)
                cB4 = cB3.rearrange("p (q b) k -> p q b k", q=NQ)
                nc.vector.tensor_tensor(
                    hc5[:, :, :, 1:31], cA4[:, :, 0:17, 1:31],
                    cB4[:, :, 0:17, 0:30], op=MIN)
                nc.scalar.copy(hc5[:, :, :, 0:1], cA4[:, :, 0:17, 0:1])

                # perc medians for this chunk's rows, via PE transpose
                percT = pool.tile([P, NQ * 512], bf16, tag="percT", name="percT")
                pT3 = percT[:].rearrange("p (q n) -> p q n", q=NQ)
                for qq in range(NQ):
                    for cg in range(4):
                        ps = ppool.tile([P, P], bf16, tag="ps", name="ps")
                        rb = kk * 256 + qq * P
                        nc.tensor.transpose(
                            ps[:], pcomp[cg][:, rb:rb + P], ident[:])
                        nc.scalar.copy(pT3[:, qq, cg * P:(cg + 1) * P], ps[:])

                # softmask: oh = S*h^2/(h^2+p^2), op = S - oh
                h2 = pool.tile([P, NQ * 512], bf16, tag="h2", name="h2")
                h23 = h2[:].rearrange("p (q n) -> p q n", q=NQ)
                nc.scalar.activation(h23, hc4[:, :, 0:512], AF.Square)
                nc.scalar.activation(percT[:], percT[:], AF.Square)
                den = pool.tile([P, NQ * 512], f32, tag="den", name="den")
                nc.vector.tensor_tensor(den[:], h2[:], percT[:], op=ADD)
                nc.vector.reciprocal(den[:], den[:])
                rb = pool.tile([P, NQ * 512], bf16, tag="rb", name="rb")
                nc.scalar.copy(rb[:], den[:])
                nc.vector.tensor_tensor(h2[:], h2[:], rb[:], op=MULT)
                nc.vector.tensor_tensor(h2[:], h2[:], xs[:], op=MULT)
                nc.vector.tensor_tensor(percT[:], xs[:], h2[:], op=SUB)
                oh_d = OH[:].rearrange("(a q p) n -> a p q n", p=P, q=NQ)[kk]
                op_d = OP[:].rearrange("(a q p) n -> a p q n", p=P, q=NQ)[kk]
                nc.sync.dma_start(oh_d, h23)
                nc.sync.dma_start(op_d, percT[:].rearrange("p (q n) -> p q n", q=NQ))

    nc.finalize()
    return nc


def _get_program():
    global _PROGRAM
    if _PROGRAM is None:
        _PROGRAM = _build_program()
    return _PROGRAM


def _host_prep(S):
    import ml_dtypes

    bf = ml_dtypes.bfloat16
    ident = np.eye(P, dtype=np.float32).astype(bf)
    # block-padded gather indices: storage slot 32*b+s <- logical 31*b+s
    bi = (K * np.arange(NB_P)[:, None] + np.arange(KS)[None, :]).reshape(-1)
    bih = (K * np.arange(NB_H)[:, None] + np.arange(KS)[None, :]).reshape(-1)
    in_maps = []
    for c in range(8):
        pl, h = c >> 1, c & 1
        b, ch = pl >> 1, pl & 1
        Sp = S[b, ch]
        # perc: transposed padded-H, logical length 35*31=1085 (+1 for ODD)
        xpl = np.zeros((512, 1086), np.float32)
        xpl[:, HALF:HALF + 1025] = Sp[:, 512 * h:512 * h + 512].T
        xp32 = xpl[:, np.minimum(bi, 1085)].astype(bf)
        xpodd = xpl[:, np.minimum(bi + 1, 1085)].astype(bf)
        # harm: per-row 558-cover (+1), logical j -> col lo+j
        lo = 512 * h - HALF
        xhl = np.zeros((1024, 559), np.float32)
        s0, s1 = max(0, lo), min(1024, lo + 559)
        xhl[:, s0 - lo:s1 - lo] = Sp[0:1024, s0:s1]
        xh32 = xhl[:, np.minimum(bih, 558)].astype(bf)
        xhodd = xhl[:, np.minimum(bih + 1, 558)].astype(bf)
        xs = Sp[0:1024, 512 * h:512 * h + 512].astype(bf)
        in_maps.append({"XP32": xp32, "XPODD": xpodd, "XH32": xh32,
                        "XHODD": xhodd, "XS": xs, "ID": ident})
    return in_maps


def _median31_rows(rows):
    """Exact median-31 along last axis with zero pad; rows [R, W]."""
    p = np.pad(rows, ((0, 0), (HALF, HALF)))
    win = np.lib.stride_tricks.sliding_window_view(p, K, axis=1)
    return np.median(win, axis=2).astype(np.float32)


def kernel(S):
    from concourse.bass_utils import run_bass_kernel_spmd

    S = np.asarray(S, np.float32)
    nc = _get_program()
    in_maps = _host_prep(S)
    res = run_bass_kernel_spmd(nc, in_maps, list(range(8)))

    out_h = np.empty_like(S)
    out_p = np.empty_like(S)
    perc_1024 = np.empty((2, 2, 1024), np.float32)
    for c in range(8):
        pl, h = c >> 1, c & 1
        b, ch = pl >> 1, pl & 1
        r = res.results[c]
        cols = slice(512 * h, 512 * h + 512)
        out_h[b, ch, 0:1024, cols] = np.asarray(r["OH"]).astype(np.float32)
        out_p[b, ch, 0:1024, cols] = np.asarray(r["OP"]).astype(np.float32)
        perc_1024[b, ch, cols] = np.asarray(r["PMR"])[:, 0].astype(np.float32)
    # host fallback: row 1024 of each plane
    rows = S[:, :, 1024, :].reshape(4, 1024)
    harm_1024 = _median31_rows(rows).reshape(2, 2, 1024)
    h2 = harm_1024 * harm_1024
    p2 = perc_1024 * perc_1024
    rden = 1.0 / (h2 + p2)
    out_h[:, :, 1024, :] = S[:, :, 1024, :] * h2 * rden
    out_p[:, :, 1024, :] = S[:, :, 1024, :] * p2 * rden
    return out_h, out_p


# revision 5
# speedup vs baseline: 2.0844x; 1.0927x over previous
"""HPSS (harmonic/percussive source separation) Trainium2 kernel, v3.

Input S [2,2,1025,1024] f32. Per (b,c) plane: harm = median-31 along W
(zero-padded), perc = median-31 along H; softmask with power=2, margin=1;
returns (S*mask_h, S*mask_p).

Sharding: 8 cores = 4 planes x 2 W-halves. Each core computes perc medians
for its 512 columns over rows 0..1024 and harm medians + softmask outputs
for rows 0..1023 x its 512 columns. Row 1024 is finished on the host.

Exact median-31 via Gil-Werman prefix/suffix order statistics. Each level
l needs its stats only on a 16-slot window per 31-block, so all level
tiles are compact [*, nb, 16]; every level's x-window is pre-gathered on
the host into a contiguous DRAM stream (XPL/XHL, 32 streams: 16 prefix +
16 suffix) so every DVE tensor op runs on fully contiguous bf16 operands
(2x mode, no strided-AP restart penalty). The merge accumulates
min-over-layers into parity-split tiles (even layers -> cminA at slot o,
odd -> cminB at o-1, both 4B-aligned) over a contiguous block range;
cross-strip/cross-q garbage blocks land in output slots nothing reads.
Perc medians stay in SBUF and are transposed with the PE.

Chunking: perc = 2 chunks x 2 column strips, harm = 2 chunks x 4 row
groups (amortizes the ~177ns fixed cost of each scan).
"""
import sys

import numpy as np

sys.path.insert(0, "/opt/trn_rl_repo")

P = 128
K = 31
KS = 32
LEV = 16
GUARD = 2.0
NB_P = 35          # perc blocks per strip (covers padded-H 1085)
NB_H = 18          # harm blocks per row group (covers 558-col strip)
NSTR = 2           # perc strips per chunk
NQ = 4             # harm row groups per chunk
NBPC = NSTR * NB_P  # 70 blocks per perc chunk
NBHC = NQ * NB_H    # 72 blocks per harm chunk
HALF = 15
NLI = 2 * LEV      # 32 level streams

_PROGRAM = None


def _build_program():
    from contextlib import ExitStack

    import concourse.mybir as mybir
    import concourse.tile as tile
    from concourse import bacc

    f32 = mybir.dt.float32
    bf16 = mybir.dt.bfloat16
    MIN = mybir.AluOpType.min
    MAX = mybir.AluOpType.max
    ADD = mybir.AluOpType.add
    MULT = mybir.AluOpType.mult
    SUB = mybir.AluOpType.subtract

    from bass_rust import ActivationFunctionType as AF

    nc = bacc.Bacc("TRN2", target_bir_lowering=False, debug=True)
    XPL = nc.declare_dram_parameter("XPL", [512, NLI * NB_P * 16], bf16,
                                    isOutput=False)
    XHL = nc.declare_dram_parameter("XHL", [1024, NLI * NB_H * 16], bf16,
                                    isOutput=False)
    XS = nc.declare_dram_parameter("XS", [1024, 512], bf16, isOutput=False)
    ID = nc.declare_dram_parameter("ID", [P, P], bf16, isOutput=False)
    OH = nc.declare_dram_parameter("OH", [1024, 512], bf16, isOutput=True)
    OP = nc.declare_dram_parameter("OP", [1024, 512], bf16, isOutput=True)
    PMR = nc.declare_dram_parameter("PMR", [512, 2], bf16, isOutput=True)

    WMAX = NBHC * 16  # 1152

    with tile.TileContext(nc) as tc:
        with ExitStack() as ctx:
            cpool = ctx.enter_context(tc.tile_pool(name="const", bufs=1))
            inpool = ctx.enter_context(tc.tile_pool(name="in", bufs=3))
            pool = ctx.enter_context(tc.tile_pool(name="work", bufs=1))
            spool = ctx.enter_context(tc.tile_pool(name="soft", bufs=1))
            ppool = ctx.enter_context(tc.tile_pool(name="ps", bufs=2,
                                                   space="PSUM"))

            mask = cpool.tile([P, WMAX], bf16)
            nc.vector.memset(mask[:], 0.0)
            nc.vector.memset(
                mask[:].rearrange("p (b k) -> p b k", k=16)[:, :, 0:1], 1e30)
            ident = cpool.tile([P, P], bf16)
            nc.sync.dma_start(ident[:], ID[:])

            pre = [pool.tile([P, WMAX], bf16, tag=f"pre{l}", name=f"pre{l}")
                   for l in range(LEV)]
            suf = [pool.tile([P, WMAX], bf16, tag=f"suf{l}", name=f"suf{l}")
                   for l in range(LEV)]

            def stream(dram, nb, j, li):
                """DMA level-stream li of chunk j into a fresh tile; returns
                the [P, W] tile (W = ngrp*nb*16)."""
                ngrp = NSTR if dram is XPL else NQ
                W = ngrp * nb * 16
                xt = inpool.tile([P, WMAX], bf16, tag="xls", name=f"xls{li}")
                src = dram[:].rearrange(
                    "(g s p) (li n) -> g li p s n", p=P, s=ngrp, li=NLI)[j, li]
                nc.sync.dma_start(
                    xt[:, 0:W].rearrange("p (s n) -> p s n", s=ngrp), src)
                return xt

            def median_chunk(dram, nb, ngrp, j):
                """Runs levels+merge for chunk j; returns (cA3, cB3) views
                [P, nbt, KS] with nbt = ngrp*nb blocks."""
                nbt = ngrp * nb
                W = nbt * 16
                mk = mask[:, 0:W]
                t = pool.tile([P, WMAX], bf16, tag="t", name="t")

                xt = stream(dram, nb, j, 0)
                nc.vector.tensor_tensor_scan(
                    pre[0][:, 0:W], mk, xt[:, 0:W], GUARD, op0=ADD, op1=MIN)
                for l in range(1, LEV):
                    xt = stream(dram, nb, j, l)
                    nc.vector.tensor_tensor(
                        t[:, 0:W], pre[l - 1][:, 0:W], xt[:, 0:W], op=MAX)
                    nc.vector.tensor_tensor_scan(
                        pre[l][:, 0:W], mk, t[:, 0:W], GUARD,
                        op0=ADD, op1=MIN)

                xt = stream(dram, nb, j, LEV)
                nc.vector.tensor_tensor_scan(
                    suf[0][:, 0:W][:, ::-1], mk, xt[:, 0:W][:, ::-1],
                    GUARD, op0=ADD, op1=MIN)
                for l in range(1, LEV):
                    xt = stream(dram, nb, j, LEV + l)
                    nc.vector.tensor_tensor(
                        t[:, 0:W], suf[l - 1][:, 0:W], xt[:, 0:W], op=MAX)
                    nc.vector.tensor_tensor_scan(
                        suf[l][:, 0:W][:, ::-1], mk, t[:, 0:W][:, ::-1],
                        GUARD, op0=ADD, op1=MIN)

                # merge over contiguous blocks 0..nbt-2 (cross-group garbage
                # blocks produce outputs nothing reads)
                nA = nbt - 1
                WA = nA * 16
                cA = pool.tile([P, NBHC * KS], bf16, tag="cA", name="cA")
                cB = pool.tile([P, NBHC * KS], bf16, tag="cB", name="cB")
                cA3 = cA[:, 0:nbt * KS].rearrange("p (b k) -> p b k", k=KS)
                cB3 = cB[:, 0:nbt * KS].rearrange("p (b k) -> p b k", k=KS)
                nc.vector.memset(cB[:, 0:nbt * KS], GUARD)
                nc.scalar.copy(
                    cA3[:, 0:nA, 0:16],
                    suf[15][:, 0:WA].rearrange("p (b k) -> p b k", k=16))
                nc.scalar.copy(
                    cA3[:, 0:nA, 16:31],
                    pre[15][:, 16:nbt * 16].rearrange(
                        "p (b k) -> p b k", k=16)[:, :, 0:15])
                tm = pool.tile([P, WMAX], bf16, tag="tm", name="tm")
                for lay in range(1, LEV):
                    nc.vector.tensor_tensor(
                        tm[:, 0:WA], suf[lay - 1][:, 0:WA],
                        pre[15 - lay][:, 16:nbt * 16], op=MAX)
                    tm3 = tm[:, 0:WA].rearrange("p (b k) -> p b k", k=16)
                    if lay % 2 == 0:
                        dst = cA3[:, 0:nA, 16 - lay:32 - lay]
                    else:
                        dst = cB3[:, 0:nA, 15 - lay:31 - lay]
                    nc.vector.tensor_tensor(dst, dst, tm3, op=MIN)
                return cA3, cB3

            # ================= perc: 2 chunks x 2 strips
            pcomp = cpool.tile([P, 4 * 1056], bf16)
            pc4 = pcomp[:].rearrange("p (g l) -> p g l", g=4)
            for j in range(2):
                cA3, cB3 = median_chunk(XPL, NB_P, NSTR, j)
                cA4 = cA3.rearrange("p (g b) k -> p g b k", g=NSTR)
                cB4 = cB3.rearrange("p (g b) k -> p g b k", g=NSTR)
                pc5 = pc4[:, 2 * j:2 * j + 2, 0:34 * K].rearrange(
                    "p g (b s) -> p g b s", s=K)
                nc.vector.tensor_tensor(
                    pc5[:, :, :, 1:31], cA4[:, :, 0:34, 1:31],
                    cB4[:, :, 0:34, 0:30], op=MIN)
                nc.scalar.copy(pc5[:, :, :, 0:1], cA4[:, :, 0:34, 0:1])
            for cg in range(4):
                nc.sync.dma_start(PMR[cg * P:(cg + 1) * P, :],
                                  pc4[:, cg, 1024:1026])

            # ================= harm: 2 chunks x 4 row groups
            for j in range(2):
                xs = inpool.tile([P, NQ * 512], bf16, tag="xs", name="xs")
                xs3 = xs[:].rearrange("p (q n) -> p q n", q=NQ)
                nc.sync.dma_start(
                    xs3, XS[:].rearrange("(a q p) n -> a p q n", p=P, q=NQ)[j])

                # perc medians for this chunk's rows via PE transpose (these
                # only depend on pcomp, so they overlap the level chains)
                percT = spool.tile([P, NQ * 512], bf16, tag=f"percT{j}",
                                   name=f"percT{j}")
                pT3 = percT[:].rearrange("p (q n) -> p q n", q=NQ)
                for qq in range(NQ):
                    for cg in range(4):
                        ps = ppool.tile([P, P], bf16, tag="ps", name="ps")
                        rb = j * 512 + qq * P
                        nc.tensor.transpose(
                            ps[:], pc4[:, cg, rb:rb + P], ident[:])
                        nc.scalar.copy(pT3[:, qq, cg * P:(cg + 1) * P], ps[:])

                cA3, cB3 = median_chunk(XHL, NB_H, NQ, j)

                hc = spool.tile([P, NQ * 527], bf16, tag="hc", name="hc")
                hc5 = hc[:].rearrange("p (q l) -> p q l", q=NQ).rearrange(
                    "p q (b s) -> p q b s", s=K)
                cA4 = cA3.rearrange("p (q b) k -> p q b k", q=NQ)
                cB4 = cB3.rearrange("p (q b) k -> p q b k", q=NQ)
                nc.vector.tensor_tensor(
                    hc5[:, :, :, 1:31], cA4[:, :, 0:17, 1:31],
                    cB4[:, :, 0:17, 0:30], op=MIN)
                nc.scalar.copy(hc5[:, :, :, 0:1], cA4[:, :, 0:17, 0:1])

                # softmask: oh = S*h^2/(h^2+p^2), op = S - oh
                hc4 = hc[:].rearrange("p (q l) -> p q l", q=NQ)
                h2 = spool.tile([P, NQ * 512], bf16, tag="h2", name="h2")
                h23 = h2[:].rearrange("p (q n) -> p q n", q=NQ)
                nc.scalar.activation(h23, hc4[:, :, 0:512], AF.Square)
                nc.scalar.activation(percT[:], percT[:], AF.Square)
                den = spool.tile([P, NQ * 512], f32, tag="den", name="den")
                nc.vector.tensor_tensor(den[:], h2[:], percT[:], op=ADD)
                nc.vector.reciprocal(den[:], den[:])
                rb16 = spool.tile([P, NQ * 512], bf16, tag="rb", name="rb")
                nc.scalar.copy(rb16[:], den[:])
                nc.vector.tensor_tensor(h2[:], h2[:], rb16[:], op=MULT)
                nc.vector.tensor_tensor(h2[:], h2[:], xs[:], op=MULT)
                nc.vector.tensor_tensor(percT[:], xs[:], h2[:], op=SUB)
                oh_d = OH[:].rearrange("(a q p) n -> a p q n", p=P, q=NQ)[j]
                op_d = OP[:].rearrange("(a q p) n -> a p q n", p=P, q=NQ)[j]
                nc.sync.dma_start(oh_d, h23)
                nc.sync.dma_start(op_d, pT3)

    nc.finalize()
    return nc


def _get_program():
    global _PROGRAM
    if _PROGRAM is None:
        _PROGRAM = _build_program()
    return _PROGRAM


def _level_idx(nb, limit):
    """[NLI, nb, 16] logical gather indices for the level streams."""
    b = K * np.arange(nb)[None, :, None]
    k = np.arange(16)[None, None, :]
    l = np.arange(LEV)[:, None, None]
    pref = l + b + k          # prefix level l: logical l+k of block b
    sufx = (HALF - l) + b + k  # suffix level l: logical 15-l+k
    idx = np.concatenate([pref, sufx], axis=0)
    return np.minimum(idx, limit)


def _host_prep(S):
    import ml_dtypes

    bf = ml_dtypes.bfloat16
    ident = np.eye(P, dtype=np.float32).astype(bf)
    pidx = _level_idx(NB_P, 1085).reshape(-1)
    hidx = _level_idx(NB_H, 558).reshape(-1)
    in_maps = []
    for c in range(8):
        pl, h = c >> 1, c & 1
        b, ch = pl >> 1, pl & 1
        Sp = S[b, ch]
        xpl = np.zeros((512, 1086), np.float32)
        xpl[:, HALF:HALF + 1025] = Sp[:, 512 * h:512 * h + 512].T
        xplb = xpl[:, pidx].astype(bf)
        lo = 512 * h - HALF
        xhl = np.zeros((1024, 559), np.float32)
        s0, s1 = max(0, lo), min(1024, lo + 559)
        xhl[:, s0 - lo:s1 - lo] = Sp[0:1024, s0:s1]
        xhlb = xhl[:, hidx].astype(bf)
        xs = Sp[0:1024, 512 * h:512 * h + 512].astype(bf)
        in_maps.append({"XPL": xplb, "XHL": xhlb, "XS": xs, "ID": ident})
    return in_maps


def _median31_rows(rows):
    p = np.pad(rows, ((0, 0), (HALF, HALF)))
    win = np.lib.stride_tricks.sliding_window_view(p, K, axis=1)
    return np.median(win, axis=2).astype(np.float32)


def kernel(S):
    from concourse.bass_utils import run_bass_kernel_spmd

    S = np.asarray(S, np.float32)
    nc = _get_program()
    in_maps = _host_prep(S)
    res = run_bass_kernel_spmd(nc, in_maps, list(range(8)))

    out_h = np.empty_like(S)
    out_p = np.empty_like(S)
    perc_1024 = np.empty((2, 2, 1024), np.float32)
    for c in range(8):
        pl, h = c >> 1, c & 1
        b, ch = pl >> 1, pl & 1
        r = res.results[c]
        cols = slice(512 * h, 512 * h + 512)
        out_h[b, ch, 0:1024, cols] = np.asarray(r["OH"]).astype(np.float32)
        out_p[b, ch, 0:1024, cols] = np.asarray(r["OP"]).astype(np.float32)
        perc_1024[b, ch, cols] = np.asarray(r["PMR"])[:, 0].astype(np.float32)
    rows = S[:, :, 1024, :].reshape(4, 1024)
    harm_1024 = _median31_rows(rows).reshape(2, 2, 1024)
    h2 = harm_1024 * harm_1024
    p2 = perc_1024 * perc_1024
    rden = 1.0 / (h2 + p2)
    out_h[:, :, 1024, :] = S[:, :, 1024, :] * h2 * rden
    out_p[:, :, 1024, :] = S[:, :, 1024, :] * p2 * rden
    return out_h, out_p


# revision 6
# speedup vs baseline: 2.1925x; 1.0518x over previous
"""HPSS (harmonic/percussive source separation) Trainium2 kernel, v3.

Input S [2,2,1025,1024] f32. Per (b,c) plane: harm = median-31 along W
(zero-padded), perc = median-31 along H; softmask with power=2, margin=1;
returns (S*mask_h, S*mask_p).

Sharding: 8 cores = 4 planes x 2 W-halves. Each core computes perc medians
for its 512 columns over rows 0..1024 and harm medians + softmask outputs
for rows 0..1023 x its 512 columns. Row 1024 is finished on the host.

Exact median-31 via Gil-Werman prefix/suffix order statistics. Each level
l needs its stats only on a 16-slot window per 31-block, so all level
tiles are compact [*, nb, 16]; every level's x-window is pre-gathered on
the host into a contiguous DRAM stream (XPL/XHL, 32 streams: 16 prefix +
16 suffix) so every DVE tensor op runs on fully contiguous bf16 operands
(2x mode, no strided-AP restart penalty). The merge accumulates
min-over-layers into parity-split tiles (even layers -> cminA at slot o,
odd -> cminB at o-1, both 4B-aligned) over a contiguous block range;
cross-strip/cross-q garbage blocks land in output slots nothing reads.
Perc medians stay in SBUF and are transposed with the PE.

Chunking: perc = 2 chunks x 2 column strips, harm = 2 chunks x 4 row
groups (amortizes the ~177ns fixed cost of each scan).
"""
import sys

import numpy as np

sys.path.insert(0, "/opt/trn_rl_repo")

P = 128
K = 31
KS = 32
LEV = 16
GUARD = 2.0
NB_P = 35          # perc blocks per strip (covers padded-H 1085)
NB_H = 18          # harm blocks per row group (covers 558-col strip)
NSTR = 2           # perc strips per chunk
NQ = 4             # harm row groups per chunk
NBPC = NSTR * NB_P  # 70 blocks per perc chunk
NBHC = NQ * NB_H    # 72 blocks per harm chunk
HALF = 15
NLI = 2 * LEV      # 32 level streams

_PROGRAM = None


def _build_program():
    from contextlib import ExitStack

    import concourse.mybir as mybir
    import concourse.tile as tile
    from concourse import bacc

    f32 = mybir.dt.float32
    bf16 = mybir.dt.bfloat16
    MIN = mybir.AluOpType.min
    MAX = mybir.AluOpType.max
    ADD = mybir.AluOpType.add
    MULT = mybir.AluOpType.mult
    SUB = mybir.AluOpType.subtract

    from bass_rust import ActivationFunctionType as AF

    nc = bacc.Bacc("TRN2", target_bir_lowering=False, debug=True)
    XPL = nc.declare_dram_parameter("XPL", [512, NLI * NB_P * 16], bf16,
                                    isOutput=False)
    XHL = nc.declare_dram_parameter("XHL", [1024, NLI * NB_H * 16], bf16,
                                    isOutput=False)
    XS = nc.declare_dram_parameter("XS", [1024, 512], bf16, isOutput=False)
    ID = nc.declare_dram_parameter("ID", [P, P], bf16, isOutput=False)
    OH = nc.declare_dram_parameter("OH", [1024, 512], bf16, isOutput=True)
    OP = nc.declare_dram_parameter("OP", [1024, 512], bf16, isOutput=True)
    PMR = nc.declare_dram_parameter("PMR", [512, 2], bf16, isOutput=True)

    WMAX = NBHC * 16  # 1152

    with tile.TileContext(nc) as tc:
        with ExitStack() as ctx:
            cpool = ctx.enter_context(tc.tile_pool(name="const", bufs=1))
            inpool = ctx.enter_context(tc.tile_pool(name="in", bufs=3))
            pool = ctx.enter_context(tc.tile_pool(name="work", bufs=1))
            spool = ctx.enter_context(tc.tile_pool(name="soft", bufs=1))
            ppool = ctx.enter_context(tc.tile_pool(name="ps", bufs=2,
                                                   space="PSUM"))

            mask = cpool.tile([P, WMAX], bf16)
            nc.vector.memset(mask[:], 0.0)
            nc.vector.memset(
                mask[:].rearrange("p (b k) -> p b k", k=16)[:, :, 0:1], 1e30)
            ident = cpool.tile([P, P], bf16)
            nc.sync.dma_start(ident[:], ID[:])

            pre = [pool.tile([P, WMAX], bf16, tag=f"pre{l}", name=f"pre{l}")
                   for l in range(LEV)]
            suf = [pool.tile([P, WMAX], bf16, tag=f"suf{l}", name=f"suf{l}")
                   for l in range(LEV)]

            def stream(dram, nb, j, li):
                """DMA level-stream li of chunk j into a fresh tile; returns
                the [P, W] tile (W = ngrp*nb*16)."""
                ngrp = NSTR if dram is XPL else NQ
                W = ngrp * nb * 16
                xt = inpool.tile([P, WMAX], bf16, tag="xls", name=f"xls{li}")
                src = dram[:].rearrange(
                    "(g s p) (li n) -> g li p s n", p=P, s=ngrp, li=NLI)[j, li]
                nc.sync.dma_start(
                    xt[:, 0:W].rearrange("p (s n) -> p s n", s=ngrp), src)
                return xt

            def median_chunk(dram, nb, ngrp, j):
                """Runs levels+merge for chunk j; returns (cA3, cB3) views
                [P, nbt, KS] with nbt = ngrp*nb blocks."""
                nbt = ngrp * nb
                W = nbt * 16
                mk = mask[:, 0:W]
                t = pool.tile([P, WMAX], bf16, tag="t", name="t")

                xt = stream(dram, nb, j, 0)
                nc.vector.tensor_tensor_scan(
                    pre[0][:, 0:W], mk, xt[:, 0:W], GUARD, op0=ADD, op1=MIN)
                for l in range(1, LEV):
                    xt = stream(dram, nb, j, l)
                    nc.vector.tensor_tensor(
                        t[:, 0:W], pre[l - 1][:, 0:W], xt[:, 0:W], op=MAX)
                    nc.vector.tensor_tensor_scan(
                        pre[l][:, 0:W], mk, t[:, 0:W], GUARD,
                        op0=ADD, op1=MIN)

                xt = stream(dram, nb, j, LEV)
                nc.vector.tensor_tensor_scan(
                    suf[0][:, 0:W][:, ::-1], mk, xt[:, 0:W][:, ::-1],
                    GUARD, op0=ADD, op1=MIN)
                for l in range(1, LEV):
                    xt = stream(dram, nb, j, LEV + l)
                    nc.vector.tensor_tensor(
                        t[:, 0:W], suf[l - 1][:, 0:W], xt[:, 0:W], op=MAX)
                    nc.vector.tensor_tensor_scan(
                        suf[l][:, 0:W][:, ::-1], mk, t[:, 0:W][:, ::-1],
                        GUARD, op0=ADD, op1=MIN)

                # merge over contiguous blocks 0..nbt-2 (cross-group garbage
                # blocks produce outputs nothing reads)
                nA = nbt - 1
                WA = nA * 16
                cA = pool.tile([P, NBHC * KS], bf16, tag="cA", name="cA")
                cB = pool.tile([P, NBHC * KS], bf16, tag="cB", name="cB")
                cA3 = cA[:, 0:nbt * KS].rearrange("p (b k) -> p b k", k=KS)
                cB3 = cB[:, 0:nbt * KS].rearrange("p (b k) -> p b k", k=KS)
                nc.vector.memset(cB[:, 0:nbt * KS], GUARD)
                nc.scalar.copy(
                    cA3[:, 0:nA, 0:16],
                    suf[15][:, 0:WA].rearrange("p (b k) -> p b k", k=16))
                nc.scalar.copy(
                    cA3[:, 0:nA, 16:31],
                    pre[15][:, 16:nbt * 16].rearrange(
                        "p (b k) -> p b k", k=16)[:, :, 0:15])
                tm = pool.tile([P, WMAX], bf16, tag="tm", name="tm")
                for lay in range(1, LEV):
                    nc.vector.tensor_tensor(
                        tm[:, 0:WA], suf[lay - 1][:, 0:WA],
                        pre[15 - lay][:, 16:nbt * 16], op=MAX)
                    tm3 = tm[:, 0:WA].rearrange("p (b k) -> p b k", k=16)
                    if lay % 2 == 0:
                        dst = cA3[:, 0:nA, 16 - lay:32 - lay]
                    else:
                        dst = cB3[:, 0:nA, 15 - lay:31 - lay]
                    nc.vector.tensor_tensor(dst, dst, tm3, op=MIN)
                return cA3, cB3

            # ================= perc: 2 chunks x 2 strips
            pcomp = cpool.tile([P, 4 * 1056], bf16)
            pc4 = pcomp[:].rearrange("p (g l) -> p g l", g=4)
            for j in range(2):
                cA3, cB3 = median_chunk(XPL, NB_P, NSTR, j)
                cA4 = cA3.rearrange("p (g b) k -> p g b k", g=NSTR)
                cB4 = cB3.rearrange("p (g b) k -> p g b k", g=NSTR)
                pc5 = pc4[:, 2 * j:2 * j + 2, 0:34 * K].rearrange(
                    "p g (b s) -> p g b s", s=K)
                nc.vector.tensor_tensor(
                    pc5[:, :, :, 1:31], cA4[:, :, 0:34, 1:31],
                    cB4[:, :, 0:34, 0:30], op=MIN)
                nc.scalar.copy(pc5[:, :, :, 0:1], cA4[:, :, 0:34, 0:1])
            for cg in range(4):
                nc.sync.dma_start(PMR[cg * P:(cg + 1) * P, :],
                                  pc4[:, cg, 1024:1026])

            # ================= harm: 2 chunks x 4 row groups
            for j in range(2):
                xs = inpool.tile([P, NQ * 512], bf16, tag="xs", name="xs")
                xs3 = xs[:].rearrange("p (q n) -> p q n", q=NQ)
                nc.sync.dma_start(
                    xs3, XS[:].rearrange("(a q p) n -> a p q n", p=P, q=NQ)[j])

                # perc medians for this chunk's rows via PE transpose (these
                # only depend on pcomp, so they overlap the level chains)
                percT = spool.tile([P, NQ * 512], bf16, tag=f"percT{j}",
                                   name=f"percT{j}")
                pT3 = percT[:].rearrange("p (q n) -> p q n", q=NQ)
                for qq in range(NQ):
                    for cg in range(4):
                        ps = ppool.tile([P, P], bf16, tag="ps", name="ps")
                        rb = j * 512 + qq * P
                        nc.tensor.transpose(
                            ps[:], pc4[:, cg, rb:rb + P], ident[:])
                        nc.scalar.copy(pT3[:, qq, cg * P:(cg + 1) * P], ps[:])

                cA3, cB3 = median_chunk(XHL, NB_H, NQ, j)

                hc = spool.tile([P, NQ * 527], bf16, tag="hc", name="hc")
                hc5 = hc[:].rearrange("p (q l) -> p q l", q=NQ).rearrange(
                    "p q (b s) -> p q b s", s=K)
                cA4 = cA3.rearrange("p (q b) k -> p q b k", q=NQ)
                cB4 = cB3.rearrange("p (q b) k -> p q b k", q=NQ)
                nc.vector.tensor_tensor(
                    hc5[:, :, :, 1:31], cA4[:, :, 0:17, 1:31],
                    cB4[:, :, 0:17, 0:30], op=MIN)
                nc.scalar.copy(hc5[:, :, :, 0:1], cA4[:, :, 0:17, 0:1])

                # softmask: oh = S*h^2/(h^2+p^2), op = S - oh
                hc4 = hc[:].rearrange("p (q l) -> p q l", q=NQ)
                h2 = spool.tile([P, NQ * 512], bf16, tag="h2", name="h2")
                h23 = h2[:].rearrange("p (q n) -> p q n", q=NQ)
                nc.scalar.activation(h23, hc4[:, :, 0:512], AF.Square)
                nc.scalar.activation(percT[:], percT[:], AF.Square)
                den = spool.tile([P, NQ * 512], f32, tag="den", name="den")
                nc.vector.tensor_tensor(den[:], h2[:], percT[:], op=ADD)
                nc.vector.reciprocal_approx_fast(den[:], den[:])
                rb16 = spool.tile([P, NQ * 512], bf16, tag="rb", name="rb")
                nc.scalar.copy(rb16[:], den[:])
                nc.vector.tensor_tensor(h2[:], h2[:], rb16[:], op=MULT)
                nc.vector.tensor_tensor(h2[:], h2[:], xs[:], op=MULT)
                nc.vector.tensor_tensor(percT[:], xs[:], h2[:], op=SUB)
                oh_d = OH[:].rearrange("(a q p) n -> a p q n", p=P, q=NQ)[j]
                op_d = OP[:].rearrange("(a q p) n -> a p q n", p=P, q=NQ)[j]
                nc.sync.dma_start(oh_d, h23)
                nc.sync.dma_start(op_d, pT3)

    nc.finalize()
    return nc


def _get_program():
    global _PROGRAM
    if _PROGRAM is None:
        _PROGRAM = _build_program()
    return _PROGRAM


def _level_idx(nb, limit):
    """[NLI, nb, 16] logical gather indices for the level streams."""
    b = K * np.arange(nb)[None, :, None]
    k = np.arange(16)[None, None, :]
    l = np.arange(LEV)[:, None, None]
    pref = l + b + k          # prefix level l: logical l+k of block b
    sufx = (HALF - l) + b + k  # suffix level l: logical 15-l+k
    idx = np.concatenate([pref, sufx], axis=0)
    return np.minimum(idx, limit)


def _host_prep(S):
    import ml_dtypes

    bf = ml_dtypes.bfloat16
    ident = np.eye(P, dtype=np.float32).astype(bf)
    pidx = _level_idx(NB_P, 1085).reshape(-1)
    hidx = _level_idx(NB_H, 558).reshape(-1)
    in_maps = []
    for c in range(8):
        pl, h = c >> 1, c & 1
        b, ch = pl >> 1, pl & 1
        Sp = S[b, ch]
        xpl = np.zeros((512, 1086), np.float32)
        xpl[:, HALF:HALF + 1025] = Sp[:, 512 * h:512 * h + 512].T
        xplb = xpl[:, pidx].astype(bf)
        lo = 512 * h - HALF
        xhl = np.zeros((1024, 559), np.float32)
        s0, s1 = max(0, lo), min(1024, lo + 559)
        xhl[:, s0 - lo:s1 - lo] = Sp[0:1024, s0:s1]
        xhlb = xhl[:, hidx].astype(bf)
        xs = Sp[0:1024, 512 * h:512 * h + 512].astype(bf)
        in_maps.append({"XPL": xplb, "XHL": xhlb, "XS": xs, "ID": ident})
    return in_maps


def _median31_rows(rows):
    p = np.pad(rows, ((0, 0), (HALF, HALF)))
    win = np.lib.stride_tricks.sliding_window_view(p, K, axis=1)
    return np.median(win, axis=2).astype(np.float32)


def kernel(S):
    from concourse.bass_utils import run_bass_kernel_spmd

    S = np.asarray(S, np.float32)
    nc = _get_program()
    in_maps = _host_prep(S)
    res = run_bass_kernel_spmd(nc, in_maps, list(range(8)))

    out_h = np.empty_like(S)
    out_p = np.empty_like(S)
    perc_1024 = np.empty((2, 2, 1024), np.float32)
    for c in range(8):
        pl, h = c >> 1, c & 1
        b, ch = pl >> 1, pl & 1
        r = res.results[c]
        cols = slice(512 * h, 512 * h + 512)
        out_h[b, ch, 0:1024, cols] = np.asarray(r["OH"]).astype(np.float32)
        out_p[b, ch, 0:1024, cols] = np.asarray(r["OP"]).astype(np.float32)
        perc_1024[b, ch, cols] = np.asarray(r["PMR"])[:, 0].astype(np.float32)
    rows = S[:, :, 1024, :].reshape(4, 1024)
    harm_1024 = _median31_rows(rows).reshape(2, 2, 1024)
    h2 = harm_1024 * harm_1024
    p2 = perc_1024 * perc_1024
    rden = 1.0 / (h2 + p2)
    out_h[:, :, 1024, :] = S[:, :, 1024, :] * h2 * rden
    out_p[:, :, 1024, :] = S[:, :, 1024, :] * p2 * rden
    return out_h, out_p


# revision 7
# speedup vs baseline: 2.2839x; 1.0417x over previous
"""HPSS (harmonic/percussive source separation) Trainium2 kernel, v5.

Input S [2,2,1025,1024] f32. Per (b,c) plane: harm = median-31 along W
(zero-padded), perc = median-31 along H; softmask with power=2, margin=1;
returns (S*mask_h, S*mask_p).

Sharding: 8 cores = 4 planes x 2 W-halves. Each core computes perc medians
for its 512 columns over rows 0..1024 and harm medians + softmask outputs
for rows 0..1023 x its 512 columns. Row 1024 is finished on the host.

Exact median-31 via Gil-Werman prefix/suffix order statistics in bf16.
Each level l only ever feeds 16 slots per 31-block downstream, so levels
live in compact [*, nb, 16] tiles; every level's x-window is pre-gathered
on the host into contiguous DRAM streams (XPL/XHL) so all DVE tensor ops
run on contiguous bf16 operands (2x mode). The layer merge accumulates
min-over-layers into parity-split tiles (even layers -> cminA at slot o,
odd -> cminB at o-1; both 4B-aligned) over a contiguous block range;
cross-strip/cross-q garbage blocks land in output slots nothing reads.
Perc medians stay in SBUF; the PE transposes them for the softmask.

v5: one mega-chunk per orientation (perc: 4 column strips stacked on the
free axis, harm: all 8 row groups) to amortize the ~180ns/scan fixed cost
and DVE dispatch gaps. Fits in SBUF by running the suffix chain first and
interleaving merge layer 15-l right after prefix level l, so only 3
rotating pre tiles are live. reciprocal_approx_fast for the softmask.
"""
import sys

import numpy as np

sys.path.insert(0, "/opt/trn_rl_repo")

P = 128
K = 31
KS = 32
LEV = 16
GUARD = 2.0
NB_P = 35          # perc blocks per strip (covers padded-H 1085)
NB_H = 18          # harm blocks per row group (covers 558-col strip)
NSTR = 4           # perc strips (all 512 cols in one chunk)
NQ = 8             # harm row groups (all 1024 rows in one chunk)
HALF = 15
NLI = 2 * LEV      # 32 level streams

_PROGRAM = None


def _build_program():
    from contextlib import ExitStack

    import concourse.mybir as mybir
    import concourse.tile as tile
    from concourse import bacc

    f32 = mybir.dt.float32
    bf16 = mybir.dt.bfloat16
    MIN = mybir.AluOpType.min
    MAX = mybir.AluOpType.max
    ADD = mybir.AluOpType.add
    MULT = mybir.AluOpType.mult
    SUB = mybir.AluOpType.subtract

    from bass_rust import ActivationFunctionType as AF

    nc = bacc.Bacc("TRN2", target_bir_lowering=False, debug=True)
    XPL = nc.declare_dram_parameter("XPL", [512, NLI * NB_P * 16], bf16,
                                    isOutput=False)
    XHL = nc.declare_dram_parameter("XHL", [1024, NLI * NB_H * 16], bf16,
                                    isOutput=False)
    XS = nc.declare_dram_parameter("XS", [1024, 512], bf16, isOutput=False)
    ID = nc.declare_dram_parameter("ID", [P, P], bf16, isOutput=False)
    OH = nc.declare_dram_parameter("OH", [1024, 512], bf16, isOutput=True)
    OP = nc.declare_dram_parameter("OP", [1024, 512], bf16, isOutput=True)
    PMR = nc.declare_dram_parameter("PMR", [512, 2], bf16, isOutput=True)

    WMAX = NQ * NB_H * 16  # 2304 compact slots (harm); perc uses 2240

    with tile.TileContext(nc) as tc:
        with ExitStack() as ctx:
            cpool = ctx.enter_context(tc.tile_pool(name="const", bufs=1))
            inpool = ctx.enter_context(tc.tile_pool(name="in", bufs=3))
            pool = ctx.enter_context(tc.tile_pool(name="work", bufs=1))
            spool = ctx.enter_context(tc.tile_pool(name="soft", bufs=1))
            ppool = ctx.enter_context(tc.tile_pool(name="ps", bufs=2,
                                                   space="PSUM"))

            mask = cpool.tile([P, WMAX], bf16)
            nc.vector.memset(mask[:], 0.0)
            nc.vector.memset(
                mask[:].rearrange("p (b k) -> p b k", k=16)[:, :, 0:1], 1e30)
            ident = cpool.tile([P, P], bf16)
            nc.sync.dma_start(ident[:], ID[:])

            suf = [pool.tile([P, WMAX], bf16, tag=f"suf{l}", name=f"suf{l}")
                   for l in range(LEV)]
            prer = [pool.tile([P, WMAX], bf16, tag=f"prer{i}", name=f"prer{i}")
                    for i in range(3)]

            def stream(dram, nb, ngrp, li):
                W = ngrp * nb * 16
                xt = inpool.tile([P, WMAX], bf16, tag="xls", name=f"xls{li}")
                src = dram[:].rearrange(
                    "(s p) (li n) -> li p s n", p=P, li=NLI)[li]
                nc.sync.dma_start(
                    xt[:, 0:W].rearrange("p (s n) -> p s n", s=ngrp), src)
                return xt

            def median_chunk(dram, nb, ngrp):
                """Suffix chain, then prefix chain with interleaved merge.
                Returns (cA3, cB3) views [P, nbt, KS]."""
                nbt = ngrp * nb
                W = nbt * 16
                nA = nbt - 1
                WA = nA * 16
                mk = mask[:, 0:W]
                t = pool.tile([P, WMAX], bf16, tag="t", name="t")
                tm = pool.tile([P, WMAX], bf16, tag="tm", name="tm")
                cA = pool.tile([P, NQ * NB_H * KS], bf16, tag="cA", name="cA")
                cB = pool.tile([P, NQ * NB_H * KS], bf16, tag="cB", name="cB")
                cA3 = cA[:, 0:nbt * KS].rearrange("p (b k) -> p b k", k=KS)
                cB3 = cB[:, 0:nbt * KS].rearrange("p (b k) -> p b k", k=KS)
                nc.vector.memset(cA[:, 0:nbt * KS], GUARD)
                nc.vector.memset(cB[:, 0:nbt * KS], GUARD)

                # ---- suffix chain: suf[l] compact k <-> logical 15-l+k
                xt = stream(dram, nb, ngrp, LEV)
                nc.vector.tensor_tensor_scan(
                    suf[0][:, 0:W][:, ::-1], mk, xt[:, 0:W][:, ::-1],
                    GUARD, op0=ADD, op1=MIN)
                for l in range(1, LEV):
                    xt = stream(dram, nb, ngrp, LEV + l)
                    nc.vector.tensor_tensor(
                        t[:, 0:W], suf[l - 1][:, 0:W], xt[:, 0:W], op=MAX)
                    nc.vector.tensor_tensor_scan(
                        suf[l][:, 0:W][:, ::-1], mk, t[:, 0:W][:, ::-1],
                        GUARD, op0=ADD, op1=MIN)
                # lay=16: pure suffix, o in [0,15]
                s153 = suf[15][:, 0:WA].rearrange("p (b k) -> p b k", k=16)
                nc.vector.tensor_tensor(
                    cA3[:, 0:nA, 0:16], cA3[:, 0:nA, 0:16], s153, op=MIN)

                # ---- prefix chain + interleaved merge layers
                xt = stream(dram, nb, ngrp, 0)
                nc.vector.tensor_tensor_scan(
                    prer[0][:, 0:W], mk, xt[:, 0:W], GUARD, op0=ADD, op1=MIN)
                for l in range(0, LEV):
                    pl = prer[l % 3]
                    if l < 15:
                        # merge layer lay = 15-l uses pre[l] & suf[14-l]
                        lay = 15 - l
                        nc.vector.tensor_tensor(
                            tm[:, 0:WA], suf[lay - 1][:, 0:WA],
                            pl[:, 16:W], op=MAX)
                        tm3 = tm[:, 0:WA].rearrange("p (b k) -> p b k", k=16)
                        if lay % 2 == 0:
                            dst = cA3[:, 0:nA, 16 - lay:32 - lay]
                        else:
                            dst = cB3[:, 0:nA, 15 - lay:31 - lay]
                        nc.vector.tensor_tensor(dst, dst, tm3, op=MIN)
                    else:
                        # lay=0: pure prefix pre[15], o in [16,30]
                        p153 = pl[:, 16:W].rearrange(
                            "p (b k) -> p b k", k=16)[:, :, 0:15]
                        nc.vector.tensor_tensor(
                            cA3[:, 0:nA, 16:31], cA3[:, 0:nA, 16:31],
                            p153, op=MIN)
                    if l < 15:
                        xt = stream(dram, nb, ngrp, l + 1)
                        nc.vector.tensor_tensor(
                            t[:, 0:W], pl[:, 0:W], xt[:, 0:W], op=MAX)
                        nc.vector.tensor_tensor_scan(
                            prer[(l + 1) % 3][:, 0:W], mk, t[:, 0:W],
                            GUARD, op0=ADD, op1=MIN)
                return cA3, cB3

            # ================= perc: one chunk, 4 strips
            pcomp = cpool.tile([P, 4 * 1056], bf16)
            pc4 = pcomp[:].rearrange("p (g l) -> p g l", g=4)
            cA3, cB3 = median_chunk(XPL, NB_P, NSTR)
            cA4 = cA3.rearrange("p (g b) k -> p g b k", g=NSTR)
            cB4 = cB3.rearrange("p (g b) k -> p g b k", g=NSTR)
            pc5 = pc4[:, :, 0:34 * K].rearrange("p g (b s) -> p g b s", s=K)
            nc.vector.tensor_tensor(
                pc5[:, :, :, 1:31], cA4[:, :, 0:34, 1:31],
                cB4[:, :, 0:34, 0:30], op=MIN)
            nc.scalar.copy(pc5[:, :, :, 0:1], cA4[:, :, 0:34, 0:1])
            for cg in range(4):
                nc.sync.dma_start(PMR[cg * P:(cg + 1) * P, :],
                                  pc4[:, cg, 1024:1026])

            # perc medians transposed for the softmask (overlaps harm chain)
            percT = spool.tile([P, NQ * 512], bf16, tag="percT", name="percT")
            pT3 = percT[:].rearrange("p (q n) -> p q n", q=NQ)
            for qq in range(NQ):
                for cg in range(4):
                    ps = ppool.tile([P, P], bf16, tag="ps", name="ps")
                    nc.tensor.transpose(
                        ps[:], pc4[:, cg, qq * P:(qq + 1) * P], ident[:])
                    nc.scalar.copy(pT3[:, qq, cg * P:(cg + 1) * P], ps[:])

            xs = spool.tile([P, NQ * 512], bf16, tag="xs", name="xs")
            xs3 = xs[:].rearrange("p (q n) -> p q n", q=NQ)
            nc.sync.dma_start(
                xs3, XS[:].rearrange("(q p) n -> p q n", p=P))

            # ================= harm: one chunk, 8 row groups
            cA3, cB3 = median_chunk(XHL, NB_H, NQ)

            hc = spool.tile([P, NQ * 527], bf16, tag="hc", name="hc")
            hc5 = hc[:].rearrange("p (q l) -> p q l", q=NQ).rearrange(
                "p q (b s) -> p q b s", s=K)
            cA4 = cA3.rearrange("p (q b) k -> p q b k", q=NQ)
            cB4 = cB3.rearrange("p (q b) k -> p q b k", q=NQ)
            nc.vector.tensor_tensor(
                hc5[:, :, :, 1:31], cA4[:, :, 0:17, 1:31],
                cB4[:, :, 0:17, 0:30], op=MIN)
            nc.scalar.copy(hc5[:, :, :, 0:1], cA4[:, :, 0:17, 0:1])

            # softmask in two halves (bounds SBUF for h2/den/rb)
            hc4 = hc[:].rearrange("p (q l) -> p q l", q=NQ)
            HQ = NQ // 2
            for half in range(2):
                qs = slice(half * HQ, (half + 1) * HQ)
                h2 = spool.tile([P, HQ * 512], bf16, tag="h2", name="h2")
                h23 = h2[:].rearrange("p (q n) -> p q n", q=HQ)
                nc.scalar.activation(h23, hc4[:, qs, 0:512], AF.Square)
                p2 = spool.tile([P, HQ * 512], bf16, tag="p2", name="p2")
                p23 = p2[:].rearrange("p (q n) -> p q n", q=HQ)
                nc.scalar.activation(p23, pT3[:, qs, :], AF.Square)
                den = spool.tile([P, HQ * 512], f32, tag="den", name="den")
                nc.vector.tensor_tensor(den[:], h2[:], p2[:], op=ADD)
                nc.vector.reciprocal_approx_fast(den[:], den[:])
                rb16 = spool.tile([P, HQ * 512], bf16, tag="rb", name="rb")
                nc.scalar.copy(rb16[:], den[:])
                xsh = xs3[:, qs, :]
                nc.vector.tensor_tensor(h23, h23, rb16[:].rearrange(
                    "p (q n) -> p q n", q=HQ), op=MULT)
                nc.vector.tensor_tensor(h23, h23, xsh, op=MULT)
                nc.vector.tensor_tensor(p23, xsh, h23, op=SUB)
                oh_d = OH[:].rearrange("(h q p) n -> h p q n", p=P, q=HQ)[half]
                op_d = OP[:].rearrange("(h q p) n -> h p q n", p=P, q=HQ)[half]
                nc.sync.dma_start(oh_d, h23)
                nc.sync.dma_start(op_d, p23)

    nc.finalize()
    return nc


def _get_program():
    global _PROGRAM
    if _PROGRAM is None:
        _PROGRAM = _build_program()
    return _PROGRAM


def _level_idx(nb, limit):
    b = K * np.arange(nb)[None, :, None]
    k = np.arange(16)[None, None, :]
    l = np.arange(LEV)[:, None, None]
    pref = l + b + k
    sufx = (HALF - l) + b + k
    idx = np.concatenate([pref, sufx], axis=0)
    return np.minimum(idx, limit)


def _host_prep(S):
    import ml_dtypes

    bf = ml_dtypes.bfloat16
    ident = np.eye(P, dtype=np.float32).astype(bf)
    pidx = _level_idx(NB_P, 1085).reshape(-1)
    hidx = _level_idx(NB_H, 558).reshape(-1)
    in_maps = []
    for c in range(8):
        pl, h = c >> 1, c & 1
        b, ch = pl >> 1, pl & 1
        Sp = S[b, ch]
        xpl = np.zeros((512, 1086), np.float32)
        xpl[:, HALF:HALF + 1025] = Sp[:, 512 * h:512 * h + 512].T
        xplb = xpl[:, pidx].astype(bf)
        lo = 512 * h - HALF
        xhl = np.zeros((1024, 559), np.float32)
        s0, s1 = max(0, lo), min(1024, lo + 559)
        xhl[:, s0 - lo:s1 - lo] = Sp[0:1024, s0:s1]
        xhlb = xhl[:, hidx].astype(bf)
        xs = Sp[0:1024, 512 * h:512 * h + 512].astype(bf)
        in_maps.append({"XPL": xplb, "XHL": xhlb, "XS": xs, "ID": ident})
    return in_maps


def _median31_rows(rows):
    p = np.pad(rows, ((0, 0), (HALF, HALF)))
    win = np.lib.stride_tricks.sliding_window_view(p, K, axis=1)
    return np.median(win, axis=2).astype(np.float32)


def kernel(S):
    from concourse.bass_utils import run_bass_kernel_spmd

    S = np.asarray(S, np.float32)
    nc = _get_program()
    in_maps = _host_prep(S)
    res = run_bass_kernel_spmd(nc, in_maps, list(range(8)))

    out_h = np.empty_like(S)
    out_p = np.empty_like(S)
    perc_1024 = np.empty((2, 2, 1024), np.float32)
    for c in range(8):
        pl, h = c >> 1, c & 1
        b, ch = pl >> 1, pl & 1
        r = res.results[c]
        cols = slice(512 * h, 512 * h + 512)
        out_h[b, ch, 0:1024, cols] = np.asarray(r["OH"]).astype(np.float32)
        out_p[b, ch, 0:1024, cols] = np.asarray(r["OP"]).astype(np.float32)
        perc_1024[b, ch, cols] = np.asarray(r["PMR"])[:, 0].astype(np.float32)
    rows = S[:, :, 1024, :].reshape(4, 1024)
    harm_1024 = _median31_rows(rows).reshape(2, 2, 1024)
    h2 = harm_1024 * harm_1024
    p2 = perc_1024 * perc_1024
    rden = 1.0 / (h2 + p2)
    out_h[:, :, 1024, :] = S[:, :, 1024, :] * h2 * rden
    out_p[:, :, 1024, :] = S[:, :, 1024, :] * p2 * rden
    return out_h, out_p


# revision 10
# speedup vs baseline: 2.3226x; 1.0170x over previous
"""HPSS (harmonic/percussive source separation) Trainium2 kernel, v5.

Input S [2,2,1025,1024] f32. Per (b,c) plane: harm = median-31 along W
(zero-padded), perc = median-31 along H; softmask with power=2, margin=1;
returns (S*mask_h, S*mask_p).

Sharding: 8 cores = 4 planes x 2 W-halves. Each core computes perc medians
for its 512 columns over rows 0..1024 and harm medians + softmask outputs
for rows 0..1023 x its 512 columns. Row 1024 is finished on the host.

Exact median-31 via Gil-Werman prefix/suffix order statistics in bf16.
Each level l only ever feeds 16 slots per 31-block downstream, so levels
live in compact [*, nb, 16] tiles; every level's x-window is pre-gathered
on the host into contiguous DRAM streams (XPL/XHL) so all DVE tensor ops
run on contiguous bf16 operands (2x mode). The layer merge accumulates
min-over-layers into parity-split tiles (even layers -> cminA at slot o,
odd -> cminB at o-1; both 4B-aligned) over a contiguous block range;
cross-strip/cross-q garbage blocks land in output slots nothing reads.
Perc medians stay in SBUF; the PE transposes them for the softmask.

v5: one mega-chunk per orientation (perc: 4 column strips stacked on the
free axis, harm: all 8 row groups) to amortize the ~180ns/scan fixed cost
and DVE dispatch gaps. Fits in SBUF by running the suffix chain first and
interleaving merge layer 15-l right after prefix level l, so only 3
rotating pre tiles are live. reciprocal_approx_fast for the softmask.
"""
import sys

import numpy as np

sys.path.insert(0, "/opt/trn_rl_repo")

P = 128
K = 31
KS = 32
LEV = 16
GUARD = 2.0
NB_P = 35          # perc blocks per strip (covers padded-H 1085)
NB_H = 18          # harm blocks per row group (covers 558-col strip)
NSTR = 4           # perc strips (all 512 cols in one chunk)
NQ = 8             # harm row groups (all 1024 rows in one chunk)
HALF = 15
NLI = 2 * LEV      # 32 level streams

_PROGRAM = None


def _build_program():
    from contextlib import ExitStack

    import concourse.mybir as mybir
    import concourse.tile as tile
    from concourse import bacc

    f32 = mybir.dt.float32
    bf16 = mybir.dt.bfloat16
    MIN = mybir.AluOpType.min
    MAX = mybir.AluOpType.max
    ADD = mybir.AluOpType.add
    MULT = mybir.AluOpType.mult
    SUB = mybir.AluOpType.subtract

    from bass_rust import ActivationFunctionType as AF

    nc = bacc.Bacc("TRN2", target_bir_lowering=False, debug=True)
    XPL = nc.declare_dram_parameter("XPL", [512, NLI * NB_P * 16], bf16,
                                    isOutput=False)
    XHL = nc.declare_dram_parameter("XHL", [1024, NLI * NB_H * 16], bf16,
                                    isOutput=False)
    XS = nc.declare_dram_parameter("XS", [1024, 512], bf16, isOutput=False)
    ID = nc.declare_dram_parameter("ID", [P, P], bf16, isOutput=False)
    GI = nc.declare_dram_parameter("GI", [P, NQ * NB_H * KS], bf16,
                                   isOutput=False)
    OH = nc.declare_dram_parameter("OH", [1024, 512], bf16, isOutput=True)
    OP = nc.declare_dram_parameter("OP", [1024, 512], bf16, isOutput=True)
    PMR = nc.declare_dram_parameter("PMR", [512, 2], bf16, isOutput=True)

    WMAX = NQ * NB_H * 16  # 2304 compact slots (harm); perc uses 2240

    with tile.TileContext(nc) as tc:
        with ExitStack() as ctx:
            cpool = ctx.enter_context(tc.tile_pool(name="const", bufs=1))
            inpool = ctx.enter_context(tc.tile_pool(name="in", bufs=3))
            pool = ctx.enter_context(tc.tile_pool(name="work", bufs=1))
            spool = ctx.enter_context(tc.tile_pool(name="soft", bufs=1))
            ppool = ctx.enter_context(tc.tile_pool(name="ps", bufs=2,
                                                   space="PSUM"))

            mask = cpool.tile([P, WMAX], bf16)
            nc.vector.memset(mask[:], 0.0)
            nc.vector.memset(
                mask[:].rearrange("p (b k) -> p b k", k=16)[:, :, 0:1], 1e30)
            ident = cpool.tile([P, P], bf16)
            nc.sync.dma_start(ident[:], ID[:])

            suf = [pool.tile([P, WMAX], bf16, tag=f"suf{l}", name=f"suf{l}")
                   for l in range(LEV)]
            prer = [pool.tile([P, WMAX], bf16, tag=f"prer{i}", name=f"prer{i}")
                    for i in range(3)]

            def stream(dram, nb, ngrp, li):
                W = ngrp * nb * 16
                xt = inpool.tile([P, WMAX], bf16, tag="xls", name=f"xls{li}")
                src = dram[:].rearrange(
                    "(s p) (li n) -> li p s n", p=P, li=NLI)[li]
                nc.sync.dma_start(
                    xt[:, 0:W].rearrange("p (s n) -> p s n", s=ngrp), src)
                return xt

            def median_chunk(dram, nb, ngrp):
                """Suffix chain, then prefix chain with interleaved merge.
                Returns (cA3, cB3) views [P, nbt, KS]."""
                nbt = ngrp * nb
                W = nbt * 16
                nA = nbt - 1
                WA = nA * 16
                mk = mask[:, 0:W]
                t = pool.tile([P, WMAX], bf16, tag="t", name="t")
                tm = pool.tile([P, WMAX], bf16, tag="tm", name="tm")
                cA = pool.tile([P, NQ * NB_H * KS], bf16, tag="cA", name="cA")
                cB = pool.tile([P, NQ * NB_H * KS], bf16, tag="cB", name="cB")
                cA3 = cA[:, 0:nbt * KS].rearrange("p (b k) -> p b k", k=KS)
                cB3 = cB[:, 0:nbt * KS].rearrange("p (b k) -> p b k", k=KS)
                nc.sync.dma_start(cA[:, 0:nbt * KS], GI[:, 0:nbt * KS])
                nc.sync.dma_start(cB[:, 0:nbt * KS], GI[:, 0:nbt * KS])

                # ---- suffix chain: suf[l] compact k <-> logical 15-l+k
                xt = stream(dram, nb, ngrp, LEV)
                nc.vector.tensor_tensor_scan(
                    suf[0][:, 0:W][:, ::-1], mk, xt[:, 0:W][:, ::-1],
                    GUARD, op0=ADD, op1=MIN)
                for l in range(1, LEV):
                    xt = stream(dram, nb, ngrp, LEV + l)
                    nc.vector.tensor_tensor(
                        t[:, 0:W], suf[l - 1][:, 0:W], xt[:, 0:W], op=MAX)
                    nc.vector.tensor_tensor_scan(
                        suf[l][:, 0:W][:, ::-1], mk, t[:, 0:W][:, ::-1],
                        GUARD, op0=ADD, op1=MIN)
                # lay=16: pure suffix, o in [0,15]
                s153 = suf[15][:, 0:WA].rearrange("p (b k) -> p b k", k=16)
                nc.vector.tensor_tensor(
                    cA3[:, 0:nA, 0:16], cA3[:, 0:nA, 0:16], s153, op=MIN)

                # ---- prefix chain + interleaved merge layers
                xt = stream(dram, nb, ngrp, 0)
                nc.vector.tensor_tensor_scan(
                    prer[0][:, 0:W], mk, xt[:, 0:W], GUARD, op0=ADD, op1=MIN)
                for l in range(0, LEV):
                    pl = prer[l % 3]
                    if l < 15:
                        # merge layer lay = 15-l uses pre[l] & suf[14-l]
                        lay = 15 - l
                        nc.vector.tensor_tensor(
                            tm[:, 0:WA], suf[lay - 1][:, 0:WA],
                            pl[:, 16:W], op=MAX)
                        tm3 = tm[:, 0:WA].rearrange("p (b k) -> p b k", k=16)
                        if lay % 2 == 0:
                            dst = cA3[:, 0:nA, 16 - lay:32 - lay]
                        else:
                            dst = cB3[:, 0:nA, 15 - lay:31 - lay]
                        nc.vector.tensor_tensor(dst, dst, tm3, op=MIN)
                    else:
                        # lay=0: pure prefix pre[15], o in [16,30]
                        p153 = pl[:, 16:W].rearrange(
                            "p (b k) -> p b k", k=16)[:, :, 0:15]
                        nc.vector.tensor_tensor(
                            cA3[:, 0:nA, 16:31], cA3[:, 0:nA, 16:31],
                            p153, op=MIN)
                    if l < 15:
                        xt = stream(dram, nb, ngrp, l + 1)
                        nc.vector.tensor_tensor(
                            t[:, 0:W], pl[:, 0:W], xt[:, 0:W], op=MAX)
                        nc.vector.tensor_tensor_scan(
                            prer[(l + 1) % 3][:, 0:W], mk, t[:, 0:W],
                            GUARD, op0=ADD, op1=MIN)
                return cA3, cB3

            # ================= perc: one chunk, 4 strips
            pcomp = cpool.tile([P, 4 * 1056], bf16)
            pc4 = pcomp[:].rearrange("p (g l) -> p g l", g=4)
            cA3, cB3 = median_chunk(XPL, NB_P, NSTR)
            cA4 = cA3.rearrange("p (g b) k -> p g b k", g=NSTR)
            cB4 = cB3.rearrange("p (g b) k -> p g b k", g=NSTR)
            pc5 = pc4[:, :, 0:34 * K].rearrange("p g (b s) -> p g b s", s=K)
            nc.vector.tensor_tensor(
                pc5[:, :, :, 1:31], cA4[:, :, 0:34, 1:31],
                cB4[:, :, 0:34, 0:30], op=MIN)
            nc.scalar.copy(pc5[:, :, :, 0:1], cA4[:, :, 0:34, 0:1])
            for cg in range(4):
                nc.sync.dma_start(PMR[cg * P:(cg + 1) * P, :],
                                  pc4[:, cg, 1024:1026])

            # perc medians transposed for the softmask (overlaps harm chain)
            percT = spool.tile([P, NQ * 512], bf16, tag="percT", name="percT")
            pT3 = percT[:].rearrange("p (q n) -> p q n", q=NQ)
            for qq in range(NQ):
                for cg in range(4):
                    ps = ppool.tile([P, P], bf16, tag="ps", name="ps")
                    nc.tensor.transpose(
                        ps[:], pc4[:, cg, qq * P:(qq + 1) * P], ident[:])
                    nc.scalar.copy(pT3[:, qq, cg * P:(cg + 1) * P], ps[:])

            xs = spool.tile([P, NQ * 512], bf16, tag="xs", name="xs")
            xs3 = xs[:].rearrange("p (q n) -> p q n", q=NQ)
            nc.sync.dma_start(
                xs3, XS[:].rearrange("(q p) n -> p q n", p=P))

            # ================= harm: one chunk, 8 row groups
            cA3, cB3 = median_chunk(XHL, NB_H, NQ)

            hc = spool.tile([P, NQ * 527], bf16, tag="hc", name="hc")
            hc5 = hc[:].rearrange("p (q l) -> p q l", q=NQ).rearrange(
                "p q (b s) -> p q b s", s=K)
            cA4 = cA3.rearrange("p (q b) k -> p q b k", q=NQ)
            cB4 = cB3.rearrange("p (q b) k -> p q b k", q=NQ)
            nc.vector.tensor_tensor(
                hc5[:, :, :, 1:31], cA4[:, :, 0:17, 1:31],
                cB4[:, :, 0:17, 0:30], op=MIN)
            nc.scalar.copy(hc5[:, :, :, 0:1], cA4[:, :, 0:17, 0:1])

            # softmask in two halves (bounds SBUF for h2/den/rb)
            hc4 = hc[:].rearrange("p (q l) -> p q l", q=NQ)
            HQ = NQ // 2
            for half in range(2):
                qs = slice(half * HQ, (half + 1) * HQ)
                h2 = spool.tile([P, HQ * 512], bf16, tag="h2", name="h2")
                h23 = h2[:].rearrange("p (q n) -> p q n", q=HQ)
                nc.scalar.activation(h23, hc4[:, qs, 0:512], AF.Square)
                p2 = spool.tile([P, HQ * 512], bf16, tag="p2", name="p2")
                p23 = p2[:].rearrange("p (q n) -> p q n", q=HQ)
                nc.scalar.activation(p23, pT3[:, qs, :], AF.Square)
                den = spool.tile([P, HQ * 512], f32, tag="den", name="den")
                nc.vector.tensor_tensor(den[:], h2[:], p2[:], op=ADD)
                nc.vector.reciprocal_approx_fast(den[:], den[:])
                rb16 = spool.tile([P, HQ * 512], bf16, tag="rb", name="rb")
                nc.scalar.copy(rb16[:], den[:])
                xsh = xs3[:, qs, :]
                nc.vector.tensor_tensor(h23, h23, rb16[:].rearrange(
                    "p (q n) -> p q n", q=HQ), op=MULT)
                nc.vector.tensor_tensor(h23, h23, xsh, op=MULT)
                nc.vector.tensor_tensor(p23, xsh, h23, op=SUB)
                oh_d = OH[:].rearrange("(h q p) n -> h p q n", p=P, q=HQ)[half]
                op_d = OP[:].rearrange("(h q p) n -> h p q n", p=P, q=HQ)[half]
                nc.sync.dma_start(oh_d, h23)
                nc.sync.dma_start(op_d, p23)

    nc.finalize()
    return nc


def _get_program():
    global _PROGRAM
    if _PROGRAM is None:
        _PROGRAM = _build_program()
    return _PROGRAM


def _level_idx(nb, limit):
    b = K * np.arange(nb)[None, :, None]
    k = np.arange(16)[None, None, :]
    l = np.arange(LEV)[:, None, None]
    pref = l + b + k
    sufx = (HALF - l) + b + k
    idx = np.concatenate([pref, sufx], axis=0)
    return np.minimum(idx, limit)


def _host_prep(S):
    import ml_dtypes

    bf = ml_dtypes.bfloat16
    ident = np.eye(P, dtype=np.float32).astype(bf)
    pidx = _level_idx(NB_P, 1085).reshape(-1)
    hidx = _level_idx(NB_H, 558).reshape(-1)
    in_maps = []
    for c in range(8):
        pl, h = c >> 1, c & 1
        b, ch = pl >> 1, pl & 1
        Sp = S[b, ch]
        xpl = np.zeros((512, 1086), np.float32)
        xpl[:, HALF:HALF + 1025] = Sp[:, 512 * h:512 * h + 512].T
        xplb = xpl[:, pidx].astype(bf)
        lo = 512 * h - HALF
        xhl = np.zeros((1024, 559), np.float32)
        s0, s1 = max(0, lo), min(1024, lo + 559)
        xhl[:, s0 - lo:s1 - lo] = Sp[0:1024, s0:s1]
        xhlb = xhl[:, hidx].astype(bf)
        xs = Sp[0:1024, 512 * h:512 * h + 512].astype(bf)
        gi = np.full((P, NQ * NB_H * KS), GUARD, np.float32).astype(bf)
        in_maps.append({"XPL": xplb, "XHL": xhlb, "XS": xs, "ID": ident,
                        "GI": gi})
    return in_maps


def _median31_rows(rows):
    p = np.pad(rows, ((0, 0), (HALF, HALF)))
    win = np.lib.stride_tricks.sliding_window_view(p, K, axis=1)
    return np.median(win, axis=2).astype(np.float32)


def kernel(S):
    from concourse.bass_utils import run_bass_kernel_spmd

    S = np.asarray(S, np.float32)
    nc = _get_program()
    in_maps = _host_prep(S)
    res = run_bass_kernel_spmd(nc, in_maps, list(range(8)))

    out_h = np.empty_like(S)
    out_p = np.empty_like(S)
    perc_1024 = np.empty((2, 2, 1024), np.float32)
    for c in range(8):
        pl, h = c >> 1, c & 1
        b, ch = pl >> 1, pl & 1
        r = res.results[c]
        cols = slice(512 * h, 512 * h + 512)
        out_h[b, ch, 0:1024, cols] = np.asarray(r["OH"]).astype(np.float32)
        out_p[b, ch, 0:1024, cols] = np.asarray(r["OP"]).astype(np.float32)
        perc_1024[b, ch, cols] = np.asarray(r["PMR"])[:, 0].astype(np.float32)
    rows = S[:, :, 1024, :].reshape(4, 1024)
    harm_1024 = _median31_rows(rows).reshape(2, 2, 1024)
    h2 = harm_1024 * harm_1024
    p2 = perc_1024 * perc_1024
    rden = 1.0 / (h2 + p2)
    out_h[:, :, 1024, :] = S[:, :, 1024, :] * h2 * rden
    out_p[:, :, 1024, :] = S[:, :, 1024, :] * p2 * rden
    return out_h, out_p


# revision 17
# speedup vs baseline: 2.3824x; 1.0257x over previous
"""HPSS (harmonic/percussive source separation) Trainium2 kernel, v5.

Input S [2,2,1025,1024] f32. Per (b,c) plane: harm = median-31 along W
(zero-padded), perc = median-31 along H; softmask with power=2, margin=1;
returns (S*mask_h, S*mask_p).

Sharding: 8 cores = 4 planes x 2 W-halves. Each core computes perc medians
for its 512 columns over rows 0..1024 and harm medians + softmask outputs
for rows 0..1023 x its 512 columns. Row 1024 is finished on the host.

Exact median-31 via Gil-Werman prefix/suffix order statistics in bf16.
Each level l only ever feeds 16 slots per 31-block downstream, so levels
live in compact [*, nb, 16] tiles; every level's x-window is pre-gathered
on the host into contiguous DRAM streams (XPL/XHL) so all DVE tensor ops
run on contiguous bf16 operands (2x mode). The layer merge accumulates
min-over-layers into parity-split tiles (even layers -> cminA at slot o,
odd -> cminB at o-1; both 4B-aligned) over a contiguous block range;
cross-strip/cross-q garbage blocks land in output slots nothing reads.
Perc medians stay in SBUF; the PE transposes them for the softmask.

v5: one mega-chunk per orientation (perc: 4 column strips stacked on the
free axis, harm: all 8 row groups) to amortize the ~180ns/scan fixed cost
and DVE dispatch gaps. Fits in SBUF by running the suffix chain first and
interleaving merge layer 15-l right after prefix level l, so only 3
rotating pre tiles are live. reciprocal_approx_fast for the softmask.
"""
import sys

import numpy as np

sys.path.insert(0, "/opt/trn_rl_repo")

P = 128
K = 31
KS = 32
LEV = 16
GUARD = 2.0
NB_P = 35          # perc blocks per strip (covers padded-H 1085)
NB_H = 18          # harm blocks per row group (covers 558-col strip)
NSTR = 4           # perc strips (all 512 cols in one chunk)
NQ = 8             # harm row groups (all 1024 rows in one chunk)
HALF = 15
NLI = 2 * LEV      # 32 level streams

_PROGRAM = None


def _build_program():
    from contextlib import ExitStack

    import concourse.mybir as mybir
    import concourse.tile as tile
    from concourse import bacc

    f32 = mybir.dt.float32
    bf16 = mybir.dt.bfloat16
    MIN = mybir.AluOpType.min
    MAX = mybir.AluOpType.max
    ADD = mybir.AluOpType.add
    MULT = mybir.AluOpType.mult
    SUB = mybir.AluOpType.subtract

    from bass_rust import ActivationFunctionType as AF

    nc = bacc.Bacc("TRN2", target_bir_lowering=False, debug=True)
    XPL = nc.declare_dram_parameter("XPL", [512, NLI * NB_P * 16], bf16,
                                    isOutput=False)
    XHL = nc.declare_dram_parameter("XHL", [1024, NLI * NB_H * 16], bf16,
                                    isOutput=False)
    XS = nc.declare_dram_parameter("XS", [1024, 512], bf16, isOutput=False)
    ID = nc.declare_dram_parameter("ID", [P, P], bf16, isOutput=False)
    GI = nc.declare_dram_parameter("GI", [P, (NQ * NB_H + 1) * KS], bf16,
                                   isOutput=False)
    OH = nc.declare_dram_parameter("OH", [1024, 512], bf16, isOutput=True)
    OP = nc.declare_dram_parameter("OP", [1024, 512], bf16, isOutput=True)
    PMR = nc.declare_dram_parameter("PMR", [512, 2], bf16, isOutput=True)

    WMAX = NQ * NB_H * 16  # 2304 compact slots (harm); perc uses 2240

    with tile.TileContext(nc) as tc:
        with ExitStack() as ctx:
            cpool = ctx.enter_context(tc.tile_pool(name="const", bufs=1))
            inpool = ctx.enter_context(tc.tile_pool(name="in", bufs=3))
            pool = ctx.enter_context(tc.tile_pool(name="work", bufs=1))
            spool = ctx.enter_context(tc.tile_pool(name="soft", bufs=1))
            ppool = ctx.enter_context(tc.tile_pool(name="ps", bufs=2,
                                                   space="PSUM"))

            mask = cpool.tile([P, WMAX], bf16)
            nc.vector.memset(mask[:], 0.0)
            nc.vector.memset(
                mask[:].rearrange("p (b k) -> p b k", k=16)[:, :, 0:1], 1e30)
            ident = cpool.tile([P, P], bf16)
            nc.sync.dma_start(ident[:], ID[:])

            suf = [pool.tile([P, WMAX], bf16, tag=f"suf{l}", name=f"suf{l}")
                   for l in range(LEV)]
            prer = [pool.tile([P, WMAX], bf16, tag=f"prer{i}", name=f"prer{i}")
                    for i in range(3)]

            def stream(dram, nb, ngrp, li):
                W = ngrp * nb * 16
                xt = inpool.tile([P, WMAX], bf16, tag="xls", name=f"xls{li}")
                src = dram[:].rearrange(
                    "(s p) (li n) -> li p s n", p=P, li=NLI)[li]
                nc.sync.dma_start(
                    xt[:, 0:W].rearrange("p (s n) -> p s n", s=ngrp), src)
                return xt

            def median_chunk(dram, nb, ngrp):
                """Suffix chain, then prefix chain with interleaved merge.
                Returns (cA3, cB3) views [P, nbt, KS]."""
                nbt = ngrp * nb
                W = nbt * 16
                nA = nbt - 1
                WA = nA * 16
                mk = mask[:, 0:W]
                t = pool.tile([P, WMAX], bf16, tag="t", name="t")
                tm = pool.tile([P, WMAX], bf16, tag="tm", name="tm")
                cA = pool.tile([P, NQ * NB_H * KS], bf16, tag="cA", name="cA")
                # cB gets a leading guard block: logical block b lives at
                # storage block b+1, so the final combine's in1 (logical
                # slot o-1, i.e. storage 32(b+1)+o-1) stays in-bounds at
                # o=0 and reads GUARD from the previous block's pad slot.
                cB = pool.tile([P, (NQ * NB_H + 1) * KS], bf16, tag="cB",
                               name="cB")
                cA3 = cA[:, 0:nbt * KS].rearrange("p (b k) -> p b k", k=KS)
                cB3 = cB[:, KS:(nbt + 1) * KS].rearrange(
                    "p (b k) -> p b k", k=KS)

                # ---- suffix chain: suf[l] compact k <-> logical 15-l+k
                xt = stream(dram, nb, ngrp, LEV)
                nc.vector.tensor_tensor_scan(
                    suf[0][:, 0:W][:, ::-1], mk, xt[:, 0:W][:, ::-1],
                    GUARD, op0=ADD, op1=MIN)
                for l in range(1, LEV):
                    xt = stream(dram, nb, ngrp, LEV + l)
                    nc.vector.tensor_tensor(
                        t[:, 0:W], suf[l - 1][:, 0:W], xt[:, 0:W], op=MAX)
                    nc.vector.tensor_tensor_scan(
                        suf[l][:, 0:W][:, ::-1], mk, t[:, 0:W][:, ::-1],
                        GUARD, op0=ADD, op1=MIN)
                # guard-init cmins (DMA, emitted after the stream DMAs so
                # the level streams win the queue; needed only at merge)
                nc.sync.dma_start(cA[:, 0:nbt * KS], GI[:, 0:nbt * KS])
                nc.sync.dma_start(cB[:, 0:(nbt + 1) * KS],
                                  GI[:, 0:(nbt + 1) * KS])
                # lay=16: pure suffix, o in [0,15]
                s153 = suf[15][:, 0:WA].rearrange("p (b k) -> p b k", k=16)
                nc.vector.tensor_tensor(
                    cA3[:, 0:nA, 0:16], cA3[:, 0:nA, 0:16], s153, op=MIN)

                # ---- prefix chain + interleaved merge layers
                xt = stream(dram, nb, ngrp, 0)
                nc.vector.tensor_tensor_scan(
                    prer[0][:, 0:W], mk, xt[:, 0:W], GUARD, op0=ADD, op1=MIN)
                for l in range(0, LEV):
                    pl = prer[l % 3]
                    if l < 15:
                        # merge layer lay = 15-l uses pre[l] & suf[14-l]
                        lay = 15 - l
                        nc.vector.tensor_tensor(
                            tm[:, 0:WA], suf[lay - 1][:, 0:WA],
                            pl[:, 16:W], op=MAX)
                        tm3 = tm[:, 0:WA].rearrange("p (b k) -> p b k", k=16)
                        if lay % 2 == 0:
                            dst = cA3[:, 0:nA, 16 - lay:32 - lay]
                        else:
                            dst = cB3[:, 0:nA, 15 - lay:31 - lay]
                        nc.vector.tensor_tensor(dst, dst, tm3, op=MIN)
                    else:
                        # lay=0: pure prefix pre[15], o in [16,30]
                        p153 = pl[:, 16:W].rearrange(
                            "p (b k) -> p b k", k=16)[:, :, 0:15]
                        nc.vector.tensor_tensor(
                            cA3[:, 0:nA, 16:31], cA3[:, 0:nA, 16:31],
                            p153, op=MIN)
                    if l < 15:
                        xt = stream(dram, nb, ngrp, l + 1)
                        nc.vector.tensor_tensor(
                            t[:, 0:W], pl[:, 0:W], xt[:, 0:W], op=MAX)
                        nc.vector.tensor_tensor_scan(
                            prer[(l + 1) % 3][:, 0:W], mk, t[:, 0:W],
                            GUARD, op0=ADD, op1=MIN)
                return cA3, cB3

            # ================= perc: one chunk, 4 strips
            pcomp = cpool.tile([P, 4 * 1056], bf16)
            pc4 = pcomp[:].rearrange("p (g l) -> p g l", g=4)
            cA3, cB3 = median_chunk(XPL, NB_P, NSTR)
            cA4 = cA3.rearrange("p (g b) k -> p g b k", g=NSTR)
            cB4 = cB3.rearrange("p (g b) k -> p g b k", g=NSTR)
            pc5 = pc4[:, :, 0:34 * K].rearrange("p g (b s) -> p g b s", s=K)
            nc.vector.tensor_tensor(
                pc5[:, :, :, 1:31], cA4[:, :, 0:34, 1:31],
                cB4[:, :, 0:34, 0:30], op=MIN)
            nc.vector.tensor_scalar_add(pc5[:, :, :, 0:1],
                                        cA4[:, :, 0:34, 0:1], 0.0)
            for cg in range(4):
                nc.sync.dma_start(PMR[cg * P:(cg + 1) * P, :],
                                  pc4[:, cg, 1024:1026])

            # perc medians transposed for the softmask (overlaps harm chain);
            # the PSUM->SBUF copy applies Square, so percT holds p^2
            percT = spool.tile([P, NQ * 512], bf16, tag="percT", name="percT")
            pT3 = percT[:].rearrange("p (q n) -> p q n", q=NQ)
            for qq in range(NQ):
                for cg in range(4):
                    ps = ppool.tile([P, P], bf16, tag="ps", name="ps")
                    nc.tensor.transpose(
                        ps[:], pc4[:, cg, qq * P:(qq + 1) * P], ident[:])
                    nc.scalar.activation(pT3[:, qq, cg * P:(cg + 1) * P],
                                         ps[:], AF.Square)

            xs = spool.tile([P, NQ * 512], bf16, tag="xs", name="xs")
            xs3 = xs[:].rearrange("p (q n) -> p q n", q=NQ)
            nc.sync.dma_start(
                xs3, XS[:].rearrange("(q p) n -> p q n", p=P))

            # ================= harm: one chunk, 8 row groups
            cA3, cB3 = median_chunk(XHL, NB_H, NQ)

            hc = spool.tile([P, NQ * 527], bf16, tag="hc", name="hc")
            hc5 = hc[:].rearrange("p (q l) -> p q l", q=NQ).rearrange(
                "p q (b s) -> p q b s", s=K)
            cA4 = cA3.rearrange("p (q b) k -> p q b k", q=NQ)
            cB4 = cB3.rearrange("p (q b) k -> p q b k", q=NQ)
            nc.vector.tensor_tensor(
                hc5[:, :, :, 1:31], cA4[:, :, 0:17, 1:31],
                cB4[:, :, 0:17, 0:30], op=MIN)
            nc.vector.tensor_scalar_add(hc5[:, :, :, 0:1],
                                        cA4[:, :, 0:17, 0:1], 0.0)

            # softmask in two halves, all on the DVE (no cross-engine hops
            # in the tail): h2 = hc*hc, den = h2+p2 (f32), r = 1/den,
            # oh = h2*r*S, op = S - oh
            hc4 = hc[:].rearrange("p (q l) -> p q l", q=NQ)
            HQ = NQ // 2
            for half in range(2):
                qs = slice(half * HQ, (half + 1) * HQ)
                h2 = spool.tile([P, HQ * 512], bf16, tag="h2", name="h2")
                h23 = h2[:].rearrange("p (q n) -> p q n", q=HQ)
                nc.vector.tensor_tensor(
                    h23, hc4[:, qs, 0:512], hc4[:, qs, 0:512], op=MULT)
                den = spool.tile([P, HQ * 512], f32, tag="den", name="den")
                nc.vector.tensor_tensor(
                    den[:], h2[:], percT[:, half * HQ * 512:
                                         (half + 1) * HQ * 512], op=ADD)
                nc.vector.reciprocal_approx_fast(den[:], den[:])
                xsh = xs3[:, qs, :]
                nc.vector.tensor_tensor(h23, h23, den[:].rearrange(
                    "p (q n) -> p q n", q=HQ), op=MULT)
                nc.vector.tensor_tensor(h23, h23, xsh, op=MULT)
                nc.vector.tensor_tensor(pT3[:, qs, :], xsh, h23, op=SUB)
                oh_d = OH[:].rearrange("(h q p) n -> h p q n", p=P, q=HQ)[half]
                op_d = OP[:].rearrange("(h q p) n -> h p q n", p=P, q=HQ)[half]
                nc.sync.dma_start(oh_d, h23)
                nc.sync.dma_start(op_d, pT3[:, qs, :])

    nc.finalize()
    return nc


def _get_program():
    global _PROGRAM
    if _PROGRAM is None:
        _PROGRAM = _build_program()
    return _PROGRAM


def _level_idx(nb, limit):
    b = K * np.arange(nb)[None, :, None]
    k = np.arange(16)[None, None, :]
    l = np.arange(LEV)[:, None, None]
    pref = l + b + k
    sufx = (HALF - l) + b + k
    idx = np.concatenate([pref, sufx], axis=0)
    return np.minimum(idx, limit)


def _host_prep(S):
    import ml_dtypes

    bf = ml_dtypes.bfloat16
    ident = np.eye(P, dtype=np.float32).astype(bf)
    pidx = _level_idx(NB_P, 1085).reshape(-1)
    hidx = _level_idx(NB_H, 558).reshape(-1)
    in_maps = []
    for c in range(8):
        pl, h = c >> 1, c & 1
        b, ch = pl >> 1, pl & 1
        Sp = S[b, ch]
        xpl = np.zeros((512, 1086), np.float32)
        xpl[:, HALF:HALF + 1025] = Sp[:, 512 * h:512 * h + 512].T
        xplb = xpl[:, pidx].astype(bf)
        lo = 512 * h - HALF
        xhl = np.zeros((1024, 559), np.float32)
        s0, s1 = max(0, lo), min(1024, lo + 559)
        xhl[:, s0 - lo:s1 - lo] = Sp[0:1024, s0:s1]
        xhlb = xhl[:, hidx].astype(bf)
        xs = Sp[0:1024, 512 * h:512 * h + 512].astype(bf)
        gi = np.full((P, (NQ * NB_H + 1) * KS), GUARD, np.float32).astype(bf)
        in_maps.append({"XPL": xplb, "XHL": xhlb, "XS": xs, "ID": ident,
                        "GI": gi})
    return in_maps


def _median31_rows(rows):
    p = np.pad(rows, ((0, 0), (HALF, HALF)))
    win = np.lib.stride_tricks.sliding_window_view(p, K, axis=1)
    return np.median(win, axis=2).astype(np.float32)


def kernel(S):
    from concourse.bass_utils import run_bass_kernel_spmd

    S = np.asarray(S, np.float32)
    nc = _get_program()
    in_maps = _host_prep(S)
    res = run_bass_kernel_spmd(nc, in_maps, list(range(8)))

    out_h = np.empty_like(S)
    out_p = np.empty_like(S)
    perc_1024 = np.empty((2, 2, 1024), np.float32)
    for c in range(8):
        pl, h = c >> 1, c & 1
        b, ch = pl >> 1, pl & 1
        r = res.results[c]
        cols = slice(512 * h, 512 * h + 512)
        out_h[b, ch, 0:1024, cols] = np.asarray(r["OH"]).astype(np.float32)
        out_p[b, ch, 0:1024, cols] = np.asarray(r["OP"]).astype(np.float32)
        perc_1024[b, ch, cols] = np.asarray(r["PMR"])[:, 0].astype(np.float32)
    rows = S[:, :, 1024, :].reshape(4, 1024)
    harm_1024 = _median31_rows(rows).reshape(2, 2, 1024)
    h2 = harm_1024 * harm_1024
    p2 = perc_1024 * perc_1024
    rden = 1.0 / (h2 + p2)
    out_h[:, :, 1024, :] = S[:, :, 1024, :] * h2 * rden
    out_p[:, :, 1024, :] = S[:, :, 1024, :] * p2 * rden
    return out_h, out_p
